# revision 30
# baseline (speedup 1.0000x reference)
"""ViT-Base + per-sample MoE adapters on 8 TRN2 NeuronCores.

Sharding: data-parallel over batch (4 samples/core, zero collectives).
Device layout: feature-major activations xT[d, t] (d on partitions, 6 chunks
of 128), bf16 matmul operands, fp32 residual. Scores are computed transposed
(sT[j,i]) so the softmax reduction becomes a ones-matmul and no on-chip
transposes are needed anywhere. LN gamma/beta and layer-scale are folded into
weights on the host; the adapter expert gather happens on the host during
sharding (it is per-sample indexing, i.e. data movement, not compute).
"""

import os
import sys

sys.path.insert(0, "/opt/trn_rl_repo")
sys.path.insert(0, "/root/.axon_site/_ro/trn_rl_repo")

from contextlib import ExitStack

import numpy as np
import ml_dtypes
from einops import rearrange

import concourse.bass as bass
import concourse.tile as tile
import concourse.mybir as mybir
from concourse import bacc
from concourse import bass_utils as _bu
from concourse.bass_utils import run_bass_kernel_spmd

# Restrict the ACT table sets to the two this kernel needs
# (natural_log_exp_and_others for LN rstd + softmax, gelu_and_others for MLP).
# With the full catalog, walrus bounces through extra sets on Square/Copy ops
# and the kernel pays ~139 table loads instead of ~26 (2.7us each, and they
# stall psum evictions). Graceful fallback to the stock file on any error.
def _setup_act_tables():
    try:
        import glob
        import json
        import tempfile
        from neuronxcc.driver.Job import Job

        cands = glob.glob(os.path.join(Job.getPackageDir(), "pwp",
                                       "pwp_bin_trainium*", "act_info.json"))
        if not cands:
            return
        src = cands[0]
        with open(src) as f:
            d = json.load(f)
        keep = {"natural_log_exp_and_others", "gelu_and_others"}
        d["act_func_sets"] = [s for s in d["act_func_sets"] if s["name"] in keep]
        if len(d["act_func_sets"]) != 2:
            return
        dstdir = tempfile.mkdtemp(prefix="act_custom_")
        import shutil

        srcdir = os.path.dirname(src)
        for fn in os.listdir(srcdir):
            if fn.endswith((".bin", ".json")) and fn != os.path.basename(src):
                try:
                    os.symlink(os.path.join(srcdir, fn), os.path.join(dstdir, fn))
                except OSError:
                    shutil.copy(os.path.join(srcdir, fn), os.path.join(dstdir, fn))
        dst = os.path.join(dstdir, os.path.basename(src))
        with open(dst, "w") as f:
            json.dump(d, f)
        os.environ["BASS_ACT_ROOT_JSON_PATH"] = dst
    except Exception:
        pass


_setup_act_tables()

# walrus's --enable-ldw-opt dedupes *consecutive identical* stationary-weight
# loads; the matmul loops below are ordered so each lhsT tile is used by two
# back-to-back matmuls (tb pairs), which halves LDWEIGHTS traffic there.
if not getattr(_bu.subprocess, "_ldwopt_patched", False):
    _orig_check_call = _bu.subprocess.check_call

    def _cc(argv, *a, **kw):
        if isinstance(argv, list) and argv and "walrus" in str(argv[0]):
            argv = ["--enable-ldw-opt=true" if x == "--enable-ldw-opt=false" else x
                    for x in argv]
        return _orig_check_call(argv, *a, **kw)

    _bu.subprocess.check_call = _cc
    _bu.subprocess._ldwopt_patched = True

# bass pre-places the table loads itself (bacc.insert_act_table_loads) using
# hw_specs.get_activation_tables; filter it to the same two sets so the
# pre-placed act_func_set_ids match the trimmed act_info.json walrus sees.
if "BASS_ACT_ROOT_JSON_PATH" in os.environ:
    from concourse import hw_specs as _hw

    _KEEP_SETS = ("natural_log_exp_and_others", "gelu_and_others")
    _orig_gat = _hw.get_activation_tables

    def _gat(arch):
        d = _orig_gat(arch)
        f = {k: d[k] for k in d if k in _KEEP_SETS}
        return f if len(f) == 2 else d

    _hw.get_activation_tables = _gat
    bacc.get_activation_tables = _gat

F32 = mybir.dt.float32
BF16 = mybir.dt.bfloat16
FP8 = mybir.dt.float8e4
AF = mybir.ActivationFunctionType
ALU = mybir.AluOpType
BF = ml_dtypes.bfloat16
# fp8 weights measured: per-layer quantization error is systematic (fixed
# weights), accumulates linearly over 12 layers -> 6e-2 rel err. Keep bf16.
FP8_MLP = False
TPAD = 800           # 16-byte-aligned token stride for fp8 pair layouts
DRMM = mybir.MatmulPerfMode.DoubleRow

B, IMG, PP, CIN = 32, 224, 16, 3
D, H, L, A, E, DD, FF = 768, 12, 12, 6, 8, 64, 3072
G = IMG // PP         # 14
N = G * G + 1         # 197
HD = D // H           # 64
NCORES = 8
S = B // NCORES       # 4 samples per core
T = S * N             # 788 tokens per core
DC = D // 128         # 6 chunks
QKC = 12              # q(6) + k(6) feature chunks
FJ = FF // 128        # 24
PC = (CIN * PP * PP) // 128  # 18
NPATCH = G * G        # 196
TB = 2
TBW = T // TB         # 394
EPS = 1e-6

_CACHE = {}


def _f(x):
    return np.asarray(x, np.float32)


def _prep(inputs):
    """Host-side prep: im2col, LN/LS folds, expert gather, bf16 packs."""
    pw = _f(inputs["patch_w"]); pb = _f(inputs["patch_b"])
    cls = _f(inputs["cls_token"]); pos = _f(inputs["pos_embed"])
    l1g = _f(inputs["ln1_g"]); l1b = _f(inputs["ln1_b"])
    qkvw = _f(inputs["qkv_w"]); qkvb = _f(inputs["qkv_b"])
    pjw = _f(inputs["proj_w"]); pjb = _f(inputs["proj_b"])
    ls1 = _f(inputs["ls1"]); ls2 = _f(inputs["ls2"])
    l2g = _f(inputs["ln2_g"]); l2b = _f(inputs["ln2_b"])
    f1w = _f(inputs["fc1_w"]); f1b = _f(inputs["fc1_b"])
    f2w = _f(inputs["fc2_w"]); f2b = _f(inputs["fc2_b"])
    ng = _f(inputs["norm_g"]); nb = _f(inputs["norm_b"])
    adw = _f(inputs["ad_down_w"]); adb = _f(inputs["ad_down_b"])
    auw = _f(inputs["ad_up_w"]); aub = _f(inputs["ad_up_b"])
    eids = np.asarray(inputs["expert_ids"], np.int64)
    imgs = _f(inputs["inputs"])

    shared = {}
    qw = qkvw[:, :, :D]; kw = qkvw[:, :, D:2 * D]; vw = qkvw[:, :, 2 * D:]
    wqk = np.concatenate([qw, kw], axis=2) * l1g[:, :, None]          # [L,768,1536]
    shared["wqk"] = rearrange(wqk, "l (c pc) (j pj) -> l j pc c pj", pc=128, pj=128).astype(BF)
    qkb = np.einsum("ldk,ld->lk", np.concatenate([qw, kw], axis=2), l1b) + qkvb[:, :2 * D]
    shared["qkb"] = rearrange(qkb, "l (j pj) -> l pj j", pj=128).astype(np.float32)

    wv = vw * l1g[:, :, None]
    shared["wv"] = rearrange(wv, "l (c pc) d -> l pc c d", pc=128).astype(BF)
    vb = np.einsum("ldk,ld->lk", vw, l1b) + qkvb[:, 2 * D:]           # [L,768]

    wproj = pjw * ls1[:, None, :]
    shared["wproj"] = rearrange(wproj, "l (c pc) (j pj) -> l j pc c pj", pc=128, pj=128).astype(BF)
    pbe = ls1 * (pjb + np.einsum("ldk,ld->lk", pjw, vb))
    shared["pbe"] = rearrange(pbe, "l (j pj) -> l pj j", pj=128).astype(np.float32)

    wfc1 = f1w * l2g[:, :, None]
    shared["wfc1"] = rearrange(wfc1, "l (c pc) (j pj) -> l j pc c pj", pc=128, pj=128).astype(BF)
    f1be = np.einsum("ldk,ld->lk", f1w, l2b) + f1b
    shared["f1be"] = rearrange(f1be, "l (j pj) -> l pj j", pj=128).astype(np.float32)

    wfc2 = f2w * ls2[:, None, :]
    shared["wfc2"] = rearrange(wfc2, "l (c pc) (j pj) -> l j pc c pj", pc=128, pj=128).astype(BF)
    f2be = ls2 * f2b
    shared["f2be"] = rearrange(f2be, "l (j pj) -> l pj j", pj=128).astype(np.float32)

    wpatch = pw.T  # [2304, 768]
    shared["wpatch"] = rearrange(wpatch, "(c pc) (j pj) -> c pc j pj", pc=128, pj=128).astype(BF)

    posb = pos[0].copy()                  # [197, 768]
    posb[1:] += pb[None, :]
    posb[0] += cls[0, 0]
    shared["posbias"] = rearrange(posb, "t (c pc) -> pc c t", pc=128).astype(np.float32)

    shared["wnorm"] = np.stack([
        rearrange(ng, "(c pc) -> pc c", pc=128),
        rearrange(nb, "(c pc) -> pc c", pc=128)], axis=-1).astype(np.float32)  # [128,6,2]

    flags = dict(
        has_pbe=bool(np.abs(pbe).max() > 0),
        has_f2be=bool(np.abs(f2be).max() > 0),
        has_qkb=bool(np.abs(qkb).max() > 0),
        has_adb=bool(np.abs(adb).max() > 0),
        has_aub=bool(np.abs(aub).max() > 0),
    )

    per_core = []
    for core in range(NCORES):
        sl = slice(core * S, (core + 1) * S)
        im = imgs[sl]
        xp = im.reshape(S, CIN, G, PP, G, PP).transpose(0, 2, 4, 1, 3, 5).reshape(
            S * NPATCH, CIN * PP * PP)
        xpT = rearrange(np.ascontiguousarray(xp.T), "(c pc) t -> c pc t", pc=128).astype(BF)
        eid = eids[sl]
        pc_map = {
            "xpT": xpT,                                              # [18,128,784]
            "adwg": rearrange(adw[:, eid], "a s (c pc) k -> a pc c s k", pc=128).astype(BF),
            "auwg": rearrange(auw[:, eid], "a s k d -> a k s d").astype(BF),   # [A,64,S,768]
            "adbg": rearrange(adb[:, eid], "a s k -> a k s").astype(np.float32),
            "aubg": rearrange(aub[:, eid], "a s (j pj) -> a s pj j", pj=128).astype(np.float32),
        }
        per_core.append(pc_map)
    return shared, per_core, flags


def _build(flags, n_layers=L, dbg=False):
    key = (tuple(sorted(flags.items())), n_layers, dbg)
    if key in _CACHE:
        return _CACHE[key]
    nc = bacc.Bacc("TRN2", target_bir_lowering=False, debug=False, num_devices=NCORES)

    def din(name, shape, dt):
        return nc.dram_tensor(name, list(shape), dt, kind="ExternalInput").ap()

    xpT_d = din("xpT", [PC, 128, S * NPATCH], BF16)
    posb_d = din("posbias", [128, DC, N], F32)
    wqk_d = din("wqk", [L, QKC, 128, DC, 128], BF16)
    qkb_d = din("qkb", [L, 128, QKC], F32)
    wv_d = din("wv", [L, 128, DC, D], BF16)
    wproj_d = din("wproj", [L, DC, 128, DC, 128], BF16)
    pbe_d = din("pbe", [L, 128, DC], F32)
    wfc1_d = din("wfc1", [L, FJ, 128, DC, 128], BF16)
    f1be_d = din("f1be", [L, 128, FJ], F32)
    wfc2_d = din("wfc2", [L, DC, 128, FJ, 128], BF16)
    f2be_d = din("f2be", [L, 128, DC], F32)
    wpatch_d = din("wpatch", [PC, 128, DC, 128], BF16)
    wnorm_d = din("wnorm", [128, DC, 2], F32)
    adwg_d = din("adwg", [A, 128, DC, S, DD], BF16)
    auwg_d = din("auwg", [A, DD, S, D], BF16)
    adbg_d = din("adbg", [A, DD, S], F32)
    aubg_d = din("aubg", [A, S, 128, DC], F32)

    out_d = nc.dram_tensor("out", [S, D], F32, kind="ExternalOutput")
    if dbg:
        xdbg_d = nc.dram_tensor("xdbg", [128, DC, T], F32, kind="ExternalOutput").ap()

    scol = [slice(s * N, (s + 1) * N) for s in range(S)]
    tbcol = [slice(tb * TBW, (tb + 1) * TBW) for tb in range(TB)]
    jts = [(0, 128), (128, N)]

    with tile.TileContext(nc) as tc:
        with ExitStack() as ctx:
            per = ctx.enter_context(tc.tile_pool(name="per", bufs=1))
            wq_p = ctx.enter_context(tc.tile_pool(name="wq", bufs=3))
            wf2_p = ctx.enter_context(tc.tile_pool(name="wf2", bufs=2))
            xp_p = ctx.enter_context(tc.tile_pool(name="xp", bufs=2))
            ad_p = ctx.enter_context(tc.tile_pool(name="ad", bufs=1))
            bia_p = ctx.enter_context(tc.tile_pool(name="bia", bufs=2))
            st_p = ctx.enter_context(tc.tile_pool(name="st", bufs=3))
            stp4 = ctx.enter_context(tc.tile_pool(name="stp4", bufs=4))
            exp_p = ctx.enter_context(tc.tile_pool(name="exp", bufs=4))
            lno_p = ctx.enter_context(tc.tile_pool(name="lno", bufs=2))
            sq_p = ctx.enter_context(tc.tile_pool(name="sq", bufs=1))
            xbf_p = ctx.enter_context(tc.tile_pool(name="xbf", bufs=2))
            ps_mm = ctx.enter_context(tc.tile_pool(name="psmm", bufs=3, space="PSUM"))
            ps_at = ctx.enter_context(tc.tile_pool(name="psat", bufs=3, space="PSUM"))
            ps_sm = ctx.enter_context(tc.tile_pool(name="pssm", bufs=2, space="PSUM"))

            x = per.tile([128, DC, T], F32, tag="x")
            qk = per.tile([128, QKC, T], BF16, tag="qk")
            v_tok = per.tile([128, S, 2, D], BF16, tag="vtok")
            attn = per.tile([128, DC, T], BF16, tag="attn")
            hml = per.tile([128, FJ, T], BF16, tag="hml")
            wv_t = per.tile([128, DC, D], BF16, tag="wvt")
            posb = per.tile([128, DC, N], F32, tag="posb")
            ones1 = per.tile([128, 1], BF16, tag="ones1")
            wnorm_t = per.tile([128, DC, 2], F32, tag="wnormt")

            nc.vector.memset(ones1[:], 1.0)
            czero = per.tile([128, 1], F32, tag="czero")
            nc.vector.memset(czero[:], 0.0)
            ceps = per.tile([128, 1], F32, tag="ceps")
            nc.vector.memset(ceps[:], EPS)
            nc.const_aps.aps[(F32, 0.0)] = czero[:]
            nc.const_aps.aps[(F32, EPS)] = ceps[:]
            nc.sync.dma_start(out=posb[:], in_=posb_d[:])
            nc.sync.dma_start(out=wnorm_t[:], in_=wnorm_d[:])

            # ======== patch embed ========
            # c-outer / j-inner: each xp chunk is DMA'd once per sample-half
            # and feeds all 6 output chunks held in 6 live psum banks
            # (3 from ps_mm + 3 from ps_at; nothing else uses psum yet).
            x_bf = xbf_p.tile([128, DC, T], BF16, tag="xbf")
            for sh in range(2):
                ps6 = []
                for j in range(DC):
                    psj = (ps_mm if j < 3 else ps_at).tile(
                        [128, 2 * NPATCH], F32, tag=("mm" if j < 3 else "at"),
                        name=f"pspe{j}")
                    ps6.append(psj)
                for c in range(PC):
                    wpc = wq_p.tile([128, DC, 128], BF16, tag="wq")
                    nc.sync.dma_start(out=wpc[:], in_=wpatch_d[c])
                    xpc = xp_p.tile([128, 2 * NPATCH], BF16, tag="xp")
                    nc.sync.dma_start(
                        out=xpc[:],
                        in_=xpT_d[c, :, sh * 2 * NPATCH:(sh + 1) * 2 * NPATCH])
                    for j in range(DC):
                        nc.tensor.matmul(ps6[j][:], wpc[:, j, :], xpc[:],
                                         start=(c == 0), stop=(c == PC - 1))
                for j in range(DC):
                    for si in range(2):
                        s = sh * 2 + si
                        nc.vector.tensor_tensor(
                            x[:, j, s * N + 1:(s + 1) * N],
                            ps6[j][:, si * NPATCH:(si + 1) * NPATCH],
                            posb[:, j, 1:N], ALU.add)
                        nc.vector.tensor_copy(x_bf[:, j, s * N + 1:(s + 1) * N],
                                              x[:, j, s * N + 1:(s + 1) * N])
            for j in range(DC):
                for s in range(S):
                    nc.vector.tensor_copy(x[:, j, s * N:s * N + 1], posb[:, j, 0:1])
                    nc.vector.tensor_copy(x_bf[:, j, s * N:s * N + 1], posb[:, j, 0:1])

            def layernorm_stats_tb(src_bf, sqt, tb):
                """One token-block's LN stats -> broadcast ab tile [128,2,TBW].
                Emitted at the tail of the producing matmul stage (proj/fc2)
                so the rstd scalar/vector/gpsimd chain overlaps the other
                token-block's PE work."""
                for c in range(DC):
                    nc.scalar.activation(sqt[:, c, :], src_bf[:, c, tbcol[tb]], AF.Square)
                sm_x = ps_sm.tile([1, TBW], F32, tag="sm")
                sm_q = ps_sm.tile([1, TBW], F32, tag="sm")
                for c in range(DC):
                    nc.tensor.matmul(sm_x[:], ones1[:], src_bf[:, c, tbcol[tb]],
                                     start=(c == 0), stop=(c == DC - 1))
                for c in range(DC):
                    nc.tensor.matmul(sm_q[:], ones1[:], sqt[:, c, :],
                                     start=(c == 0), stop=(c == DC - 1))
                # mA = mean; varD = Sum(x^2) - Sum(x)^2/D; r = (varD/D + eps)^-1/2
                mA = stp4.tile([1, TBW], F32, tag="stat")
                nc.vector.tensor_scalar_mul(mA[:], sm_x[:], 1.0 / D)
                msqD = stp4.tile([1, TBW], F32, tag="stat")
                nc.vector.tensor_tensor(msqD[:], mA[:], sm_x[:], ALU.mult)
                varD = stp4.tile([1, TBW], F32, tag="stat")
                nc.vector.tensor_tensor(varD[:], sm_q[:], msqD[:], ALU.subtract)
                r = stp4.tile([1, TBW], F32, tag="stat")
                nc.scalar.activation(r[:], varD[:], AF.Ln, bias=EPS, scale=1.0 / D)
                ab = st_p.tile([1, 2, TBW], BF16, tag="ab")
                nc.scalar.activation(ab[:, 0, :], r[:], AF.Exp, scale=-0.5)
                mr = stp4.tile([1, TBW], F32, tag="stat")
                nc.vector.tensor_tensor(mr[:], mA[:], ab[:, 0, :], ALU.mult)
                nc.vector.tensor_scalar_mul(ab[:, 1, :], mr[:], -1.0)
                abb = st_p.tile([128, 2, TBW], BF16, tag="abb")
                nc.gpsimd.partition_broadcast(abb[:], ab[0:1, :, :])
                return abb

            def ln_apply_tb(src_bf, abb, dst, tb):
                for c in range(DC):
                    nc.vector.tensor_tensor(dst[:, c, tbcol[tb]],
                                            src_bf[:, c, tbcol[tb]],
                                            abb[:, 0, :], ALU.mult)
                    nc.vector.tensor_tensor(dst[:, c, tbcol[tb]],
                                            dst[:, c, tbcol[tb]],
                                            abb[:, 1, :], ALU.add)

            # ======== transformer layers ========
            ln1_t = None
            for l in range(n_layers):
                # ---- LN1 ---- (x_bf shadow-written by patch/fc2 evicts;
                # stats+apply for l>0 were emitted at the tail of fc2)
                if ln1_t is None:
                    sqt = sq_p.tile([128, DC, TBW], BF16, tag="sq", name="sqt0")
                    ln1 = lno_p.tile([128, DC, T], BF16, tag="lno", name="ln1_0")
                    for tb in range(TB):
                        abb0 = layernorm_stats_tb(x_bf, sqt, tb)
                        ln_apply_tb(x_bf, abb0, ln1, tb)
                else:
                    ln1 = ln1_t

                # ---- QK ---- staggered: tb0 chains run 2 j ahead of tb1 so
                # the PE never sits behind the LN1(tb1) apply; q/k chunk pairs
                # emitted together so attention head-pair a can start early.
                if flags["has_qkb"]:
                    qkb_t = bia_p.tile([128, QKC], F32, tag="qkb")
                    nc.sync.dma_start(out=qkb_t[:], in_=qkb_d[l])

                def qk_evict(j, tb, ps):
                    if flags["has_qkb"]:
                        nc.scalar.activation(qk[:, j, tbcol[tb]], ps[:], AF.Identity,
                                             bias=qkb_t[:, j:j + 1])
                    else:
                        nc.scalar.copy(qk[:, j, tbcol[tb]], ps[:])

                qk_seq = [0, 6, 1, 7, 2, 8, 3, 9, 4, 10, 5, 11]
                qk_wjs = {}
                for idx, j in enumerate(qk_seq):
                    wj = wq_p.tile([128, DC, 128], BF16, tag="wq", name=f"wqk{l}_{j}")
                    nc.sync.dma_start(out=wj[:], in_=wqk_d[l, j])
                    qk_wjs[j] = wj
                    ps0 = ps_mm.tile([128, TBW], F32, tag="mm", name=f"qkp0_{j}")
                    for c in range(DC):
                        nc.tensor.matmul(ps0[:], wj[:, c, :], ln1[:, c, tbcol[0]],
                                         start=(c == 0), stop=(c == DC - 1))
                    qk_evict(j, 0, ps0)
                    if idx >= 2:
                        jp = qk_seq[idx - 2]
                        wjp = qk_wjs.pop(jp)
                        ps1 = ps_mm.tile([128, TBW], F32, tag="mm", name=f"qkp1_{jp}")
                        for c in range(DC):
                            nc.tensor.matmul(ps1[:], wjp[:, c, :],
                                             ln1[:, c, tbcol[1]],
                                             start=(c == 0), stop=(c == DC - 1))
                        qk_evict(jp, 1, ps1)
                for j in qk_seq[-2:]:
                    wjp = qk_wjs.pop(j)
                    ps1 = ps_mm.tile([128, TBW], F32, tag="mm", name=f"qkp1t_{j}")
                    for c in range(DC):
                        nc.tensor.matmul(ps1[:], wjp[:, c, :],
                                         ln1[:, c, tbcol[1]],
                                         start=(c == 0), stop=(c == DC - 1))
                    qk_evict(j, 1, ps1)

                # ---- V (token-major) ----
                nc.sync.dma_start(out=wv_t[:], in_=wv_d[l])
                for s in range(S):
                    for jt, (j0, j1) in enumerate(jts):
                        tn = j1 - j0
                        ps0 = ps_mm.tile([128, 384], F32, tag="mm")
                        ps1 = ps_mm.tile([128, 384], F32, tag="mm")
                        for c in range(DC):
                            lh = ln1[:, c, s * N + j0:s * N + j1]
                            nc.tensor.matmul(ps0[:tn, :], lh, wv_t[:, c, 0:384],
                                             start=(c == 0), stop=(c == DC - 1))
                            nc.tensor.matmul(ps1[:tn, :], lh, wv_t[:, c, 384:768],
                                             start=(c == 0), stop=(c == DC - 1))
                        nc.vector.tensor_copy(v_tok[:tn, s, jt, 0:384], ps0[:tn, :])
                        nc.vector.tensor_copy(v_tok[:tn, s, jt, 384:768], ps1[:tn, :])

                # ---- attention ---- software-pipelined over (s, head-pair).
                # Stages: st0 scores+exp -> st1 ones-sum+recip+bcast -> st2
                # attnV -> st3 evict. Lookahead-2 emission keeps independent
                # score streams in front of the PE queue while the current
                # item's exp/recip/broadcast chain resolves on the other
                # engines.
                items = [(s, a) for s in range(S) for a in range(DC)]
                at_exp, at_rcb, at_ps = {}, {}, {}

                def att_st0(it):
                    s, a = it
                    sT_e = ps_at.tile([128, 2, N], F32, tag="at")
                    sT_o = ps_at.tile([128, 2, N], F32, tag="at")
                    # jt0+jt1 chained into one accumulation group per psum
                    # tile (disjoint halves, start=True on both) -> one
                    # group-end drain instead of two.
                    for sT, p0, tp in ((sT_e, 0, (0, 0)), (sT_o, 64, (64, 0))):
                        for jt, (j0, j1) in enumerate(jts):
                            tn = j1 - j0
                            nc.tensor.matmul(
                                sT[:tn, jt, :],
                                qk[p0:p0 + 64, DC + a, scol[s]][:, j0:j1],
                                qk[p0:p0 + 64, a, scol[s]],
                                start=True, stop=(jt == 1),
                                skip_group_check=(jt == 1),
                                tile_position=tp)
                    expe = exp_p.tile([128, 2, N], BF16, tag="exp")
                    expo = exp_p.tile([128, 2, N], BF16, tag="exp")
                    # one wide exp per head; rows 69-127 of the jt=1 slice are
                    # garbage (never read by the cs/oT matmuls below)
                    nc.scalar.activation(expe[:], sT_e[:], AF.Exp, scale=0.125)
                    nc.scalar.activation(expo[:], sT_o[:], AF.Exp, scale=0.125)
                    at_exp[it] = (expe, expo)

                def att_st1(it):
                    expe, expo = at_exp[it]
                    cs = ps_sm.tile([1, 2, N], F32, tag="sm")
                    nc.tensor.matmul(cs[:, 0, :], ones1[:], expe[:, 0, :],
                                     start=True, stop=False)
                    nc.tensor.matmul(cs[:, 0, :], ones1[:69, :], expe[:69, 1, :],
                                     start=False, stop=False)
                    nc.tensor.matmul(cs[:, 1, :], ones1[:], expo[:, 0, :],
                                     start=True, stop=False, skip_group_check=True)
                    nc.tensor.matmul(cs[:, 1, :], ones1[:69, :], expo[:69, 1, :],
                                     start=False, stop=True)
                    rec = st_p.tile([1, 2, N], F32, tag="rec", bufs=2)
                    nc.vector.reciprocal_approx_fast(rec[:], cs[:])
                    rcb = st_p.tile([128, 2, N], F32, tag="rcb", bufs=2)
                    nc.gpsimd.partition_broadcast(rcb[:], rec[0:1, :, :])
                    at_rcb[it] = rcb

                def att_st2(it):
                    s, a = it
                    expe, expo = at_exp[it]
                    psA = ps_mm.tile([128, N], F32, tag="mm")
                    psB = ps_mm.tile([128, N], F32, tag="mm")
                    dlo = a * 128
                    nc.tensor.matmul(psA[:], v_tok[:, s, 0, dlo:dlo + 128], expe[:, 0, :],
                                     start=True, stop=False)
                    nc.tensor.matmul(psB[:], v_tok[:, s, 0, dlo:dlo + 128], expo[:, 0, :],
                                     start=True, stop=False)
                    nc.tensor.matmul(psA[:], v_tok[:69, s, 1, dlo:dlo + 128],
                                     expe[:69, 1, :], start=False, stop=True)
                    nc.tensor.matmul(psB[:], v_tok[:69, s, 1, dlo:dlo + 128],
                                     expo[:69, 1, :], start=False, stop=True)
                    at_ps[it] = (psA, psB)

                def att_st3(it):
                    s, a = it
                    psA, psB = at_ps.pop(it)
                    rcb = at_rcb.pop(it)
                    at_exp.pop(it)
                    nc.vector.tensor_tensor(attn[0:64, a, scol[s]], psA[0:64, :],
                                            rcb[0:64, 0, :], ALU.mult)
                    nc.vector.tensor_tensor(attn[64:128, a, scol[s]], psB[64:128, :],
                                            rcb[64:128, 1, :], ALU.mult)

                att_st0(items[0])
                att_st0(items[1])
                att_st1(items[0])
                for i in range(len(items)):
                    if i + 2 < len(items):
                        att_st0(items[i + 2])
                    if i + 1 < len(items):
                        att_st1(items[i + 1])
                    att_st2(items[i])
                    att_st3(items[i])

                # Last layer: only the CLS columns survive to the final LN, so
                # proj/LN2/MLP run on 4 columns instead of 788.
                last = (l == n_layers - 1) and (not dbg) and l >= A
                if last:
                    if flags["has_pbe"]:
                        pbe_t = bia_p.tile([128, DC], F32, tag="pbe")
                        nc.sync.dma_start(out=pbe_t[:], in_=pbe_d[l])
                    for j in range(DC):
                        wj = wq_p.tile([128, DC, 128], BF16, tag="wq")
                        nc.sync.dma_start(out=wj[:], in_=wproj_d[l, j])
                        ps = ps_mm.tile([128, S], F32, tag="mm")
                        for c in range(DC):
                            nc.tensor.matmul(ps[:], wj[:, c, :], attn[:, c, ::N],
                                             start=(c == 0), stop=(c == DC - 1))
                        if flags["has_pbe"]:
                            nc.vector.scalar_tensor_tensor(
                                x[:, j, ::N], ps[:], pbe_t[:, j:j + 1],
                                x[:, j, ::N], ALU.add, ALU.add)
                        else:
                            nc.vector.tensor_tensor(x[:, j, ::N], ps[:],
                                                    x[:, j, ::N], ALU.add)
                    # LN2 on CLS columns
                    xbfc = st_p.tile([128, DC, S], BF16, tag="xbfc")
                    for c in range(DC):
                        nc.vector.tensor_copy(xbfc[:, c, :], x[:, c, ::N])
                    sqc2 = st_p.tile([128, DC, S], BF16, tag="sqc2")
                    nc.scalar.activation(sqc2[:], xbfc[:], AF.Square)
                    smc_x = ps_sm.tile([1, S], F32, tag="sm")
                    smc_q = ps_sm.tile([1, S], F32, tag="sm")
                    for c in range(DC):
                        nc.tensor.matmul(smc_x[:], ones1[:], xbfc[:, c, :],
                                         start=(c == 0), stop=(c == DC - 1))
                    for c in range(DC):
                        nc.tensor.matmul(smc_q[:], ones1[:], sqc2[:, c, :],
                                         start=(c == 0), stop=(c == DC - 1))
                    mAc = st_p.tile([1, S], F32, tag="mAc")
                    nc.vector.tensor_scalar_mul(mAc[:], smc_x[:], 1.0 / D)
                    msqc = st_p.tile([1, S], F32, tag="msqc")
                    nc.vector.tensor_tensor(msqc[:], mAc[:], smc_x[:], ALU.mult)
                    varc = st_p.tile([1, S], F32, tag="varc")
                    nc.vector.tensor_tensor(varc[:], smc_q[:], msqc[:], ALU.subtract)
                    rc = st_p.tile([1, S], F32, tag="rcl")
                    nc.scalar.activation(rc[:], varc[:], AF.Ln, bias=EPS, scale=1.0 / D)
                    abc = st_p.tile([1, 2, S], BF16, tag="abc")
                    nc.scalar.activation(abc[:, 0, :], rc[:], AF.Exp, scale=-0.5)
                    mrc = st_p.tile([1, S], F32, tag="mrc")
                    nc.vector.tensor_tensor(mrc[:], mAc[:], abc[:, 0, :], ALU.mult)
                    nc.vector.tensor_scalar_mul(abc[:, 1, :], mrc[:], -1.0)
                    abbc = st_p.tile([128, 2, S], BF16, tag="abbc")
                    nc.gpsimd.partition_broadcast(abbc[:], abc[0:1, :, :])
                    ln2c = st_p.tile([128, DC, S], BF16, tag="ln2c")
                    for c in range(DC):
                        nc.vector.tensor_tensor(ln2c[:, c, :], xbfc[:, c, :],
                                                abbc[:, 0, :], ALU.mult)
                        nc.vector.tensor_tensor(ln2c[:, c, :], ln2c[:, c, :],
                                                abbc[:, 1, :], ALU.add)
                    # MLP on CLS columns
                    f1be_t = bia_p.tile([128, FJ], F32, tag="f1b")
                    nc.sync.dma_start(out=f1be_t[:], in_=f1be_d[l])
                    if flags["has_f2be"]:
                        f2be_t = bia_p.tile([128, DC], F32, tag="f2b")
                        nc.sync.dma_start(out=f2be_t[:], in_=f2be_d[l])
                    hc = st_p.tile([128, FJ, S], BF16, tag="hc")
                    for j in range(FJ):
                        wj = wq_p.tile([128, DC, 128], BF16, tag="wq")
                        nc.sync.dma_start(out=wj[:], in_=wfc1_d[l, j])
                        ps = ps_mm.tile([128, S], F32, tag="mm")
                        for c in range(DC):
                            nc.tensor.matmul(ps[:], wj[:, c, :], ln2c[:, c, :],
                                             start=(c == 0), stop=(c == DC - 1))
                        nc.scalar.activation(hc[:, j, :], ps[:], AF.Gelu,
                                             bias=f1be_t[:, j:j + 1])
                    for j in range(DC):
                        w2j = wf2_p.tile([128, FJ, 128], BF16, tag="wf2")
                        nc.sync.dma_start(out=w2j[:], in_=wfc2_d[l, j])
                        ps = ps_mm.tile([128, S], F32, tag="mm")
                        for c in range(FJ):
                            nc.tensor.matmul(ps[:], w2j[:, c, :], hc[:, c, :],
                                             start=(c == 0), stop=(c == FJ - 1))
                        if flags["has_f2be"]:
                            nc.vector.scalar_tensor_tensor(
                                x[:, j, ::N], ps[:], f2be_t[:, j:j + 1],
                                x[:, j, ::N], ALU.add, ALU.add)
                        else:
                            nc.vector.tensor_tensor(x[:, j, ::N], ps[:],
                                                    x[:, j, ::N], ALU.add)
                    continue

                # ---- proj + residual (shadow bf16 for LN2/adapter) ----
                x_bf2 = xbf_p.tile([128, DC, T], BF16, tag="xbf")
                if flags["has_pbe"]:
                    pbe_t = bia_p.tile([128, DC], F32, tag="pbe")
                    nc.sync.dma_start(out=pbe_t[:], in_=pbe_d[l])
                # tb-outer so LN2 stats(tb0) + rstd + apply(tb0) overlap
                # proj(tb1) PE work (costs a second wproj DMA pass, 1.2MB).
                sqt2 = sq_p.tile([128, DC, TBW], BF16, tag="sq")
                ln2 = lno_p.tile([128, DC, T], BF16, tag="lno")
                for tb in range(TB):
                    for j in range(DC):
                        wj = wq_p.tile([128, DC, 128], BF16, tag="wq")
                        nc.sync.dma_start(out=wj[:], in_=wproj_d[l, j])
                        ps = ps_mm.tile([128, TBW], F32, tag="mm")
                        for c in range(DC):
                            nc.tensor.matmul(ps[:], wj[:, c, :], attn[:, c, tbcol[tb]],
                                             start=(c == 0), stop=(c == DC - 1))
                        if flags["has_pbe"]:
                            nc.vector.scalar_tensor_tensor(
                                x[:, j, tbcol[tb]], ps[:], pbe_t[:, j:j + 1],
                                x[:, j, tbcol[tb]], ALU.add, ALU.add)
                        else:
                            nc.vector.tensor_tensor(x[:, j, tbcol[tb]], ps[:],
                                                    x[:, j, tbcol[tb]], ALU.add)
                        nc.vector.tensor_copy(x_bf2[:, j, tbcol[tb]],
                                              x[:, j, tbcol[tb]])
                    abb2 = layernorm_stats_tb(x_bf2, sqt2, tb)
                    ln_apply_tb(x_bf2, abb2, ln2, tb)

                # ---- adapter ----
                if l < A:
                    adw_t = ad_p.tile([128, DC, S, DD], BF16, tag="adw")
                    auw_t = ad_p.tile([DD, S, D], BF16, tag="auw")
                    nc.sync.dma_start(out=adw_t[:], in_=adwg_d[l])
                    nc.sync.dma_start(out=auw_t[:], in_=auwg_d[l])
                    if flags["has_adb"]:
                        adbg_t = bia_p.tile([DD, S], F32, tag="adb")
                        nc.sync.dma_start(out=adbg_t[:], in_=adbg_d[l])
                    if flags["has_aub"]:
                        aubg_t = bia_p.tile([S, 128, DC], F32, tag="aub")
                        nc.sync.dma_start(out=aubg_t[:], in_=aubg_d[l])
                def adapter_compute(adw_t=None, auw_t=None, x_src=None,
                                    adbg=None, aubg=None):
                    # psums come from ps_at (idle during the MLP); gelus are
                    # emitted mid-fc1 so they join the gelu table-set run
                    # instead of thrashing against LN exp/ln loads.
                    for s in range(S):
                        psh = ps_at.tile([DD, N], F32, tag="at")
                        for c in range(DC):
                            nc.tensor.matmul(psh[:], adw_t[:, c, s, :], x_src[:, c, scol[s]],
                                             start=(c == 0), stop=(c == DC - 1))
                        hp = st_p.tile([DD, N], BF16, tag="hp")
                        if adbg is not None:
                            nc.scalar.activation(hp[:], psh[:], AF.Gelu,
                                                 bias=adbg[:, s:s + 1])
                        else:
                            nc.scalar.activation(hp[:], psh[:], AF.Gelu)
                        for j in range(DC):
                            psu = ps_at.tile([128, N], F32, tag="at")
                            nc.tensor.matmul(psu[:], auw_t[:, s, j * 128:(j + 1) * 128],
                                             hp[:], start=True, stop=True)
                            if aubg is not None:
                                nc.vector.scalar_tensor_tensor(
                                    x[:, j, scol[s]], psu[:], aubg[s, :, j:j + 1],
                                    x[:, j, scol[s]], ALU.add, ALU.add)
                            else:
                                nc.vector.tensor_tensor(x[:, j, scol[s]], psu[:],
                                                        x[:, j, scol[s]], ALU.add)

                if l < A:
                    ad_args = dict(adw_t=adw_t, auw_t=auw_t, x_src=x_bf2,
                                   adbg=adbg_t if flags["has_adb"] else None,
                                   aubg=aubg_t if flags["has_aub"] else None)

                # ---- MLP ---- (fc2 evicts shadow next layer's LN1 input)
                x_bf = xbf_p.tile([128, DC, T], BF16, tag="xbf")
                f1be_t = bia_p.tile([128, FJ], F32, tag="f1b")
                nc.sync.dma_start(out=f1be_t[:], in_=f1be_d[l])
                if flags["has_f2be"]:
                    f2be_t = bia_p.tile([128, DC], F32, tag="f2b")
                    nc.sync.dma_start(out=f2be_t[:], in_=f2be_d[l])
                # fc1 staggered: tb0 chains run 2 j ahead of tb1 so the PE
                # doesn't wait on the LN2(tb1) apply.
                f1_wjs = {}
                for j in range(FJ):
                    wj = wq_p.tile([128, DC, 128], BF16, tag="wq", name=f"wf1_{j}")
                    nc.sync.dma_start(out=wj[:], in_=wfc1_d[l, j])
                    f1_wjs[j] = wj
                    ps0 = ps_mm.tile([128, TBW], F32, tag="mm", name=f"f1p0_{j}")
                    for c in range(DC):
                        nc.tensor.matmul(ps0[:], wj[:, c, :], ln2[:, c, tbcol[0]],
                                         start=(c == 0), stop=(c == DC - 1))
                    nc.scalar.activation(hml[:, j, tbcol[0]], ps0[:], AF.Gelu,
                                         bias=f1be_t[:, j:j + 1])
                    if j >= 2:
                        wjp = f1_wjs.pop(j - 2)
                        ps1 = ps_mm.tile([128, TBW], F32, tag="mm", name=f"f1p1_{j}")
                        for c in range(DC):
                            nc.tensor.matmul(ps1[:], wjp[:, c, :],
                                             ln2[:, c, tbcol[1]],
                                             start=(c == 0), stop=(c == DC - 1))
                        nc.scalar.activation(hml[:, j - 2, tbcol[1]], ps1[:], AF.Gelu,
                                             bias=f1be_t[:, j - 2:j - 1])
                    if j == 5 and l < A:
                        adapter_compute(**ad_args)
                for j in (FJ - 2, FJ - 1):
                    wjp = f1_wjs.pop(j)
                    ps1 = ps_mm.tile([128, TBW], F32, tag="mm", name=f"f1p1t_{j}")
                    for c in range(DC):
                        nc.tensor.matmul(ps1[:], wjp[:, c, :],
                                         ln2[:, c, tbcol[1]],
                                         start=(c == 0), stop=(c == DC - 1))
                    nc.scalar.activation(hml[:, j, tbcol[1]], ps1[:], AF.Gelu,
                                         bias=f1be_t[:, j:j + 1])
                # fc2 tb-outer: LN1(l+1) stats+apply for each tb run right
                # after its last evict; the rstd chain overlaps the other
                # tb's (or QK's) PE work.
                sqt_n = sq_p.tile([128, DC, TBW], BF16, tag="sq", name=f"sqtn{l}")
                ln1_t = lno_p.tile([128, DC, T], BF16, tag="lno", name=f"ln1n{l}")
                for tb in range(TB):
                    for j in range(DC):
                        w2j = wf2_p.tile([128, FJ, 128], BF16, tag="wf2",
                                         name=f"w2j{tb}_{j}")
                        nc.sync.dma_start(out=w2j[:], in_=wfc2_d[l, j])
                        ps = ps_mm.tile([128, TBW], F32, tag="mm")
                        for c in range(FJ):
                            nc.tensor.matmul(ps[:], w2j[:, c, :], hml[:, c, tbcol[tb]],
                                             start=(c == 0), stop=(c == FJ - 1))
                        if flags["has_f2be"]:
                            nc.vector.scalar_tensor_tensor(
                                x[:, j, tbcol[tb]], ps[:], f2be_t[:, j:j + 1],
                                x[:, j, tbcol[tb]], ALU.add, ALU.add)
                        else:
                            nc.vector.tensor_tensor(x[:, j, tbcol[tb]], ps[:],
                                                    x[:, j, tbcol[tb]], ALU.add)
                        nc.vector.tensor_copy(x_bf[:, j, tbcol[tb]],
                                              x[:, j, tbcol[tb]])
                    abbn = layernorm_stats_tb(x_bf, sqt_n, tb)
                    ln_apply_tb(x_bf, abbn, ln1_t, tb)

            if dbg:
                for c in range(DC):
                    nc.sync.dma_start(out=xdbg_d[:, c, :], in_=x[:, c, :])

            # ======== final LN on CLS columns + output ========
            xc = st_p.tile([128, DC, S], F32, tag="xc")
            for c in range(DC):
                nc.vector.tensor_copy(xc[:, c, :], x[:, c, ::N])
            xcb = st_p.tile([128, DC, S], BF16, tag="xcb")
            nc.vector.tensor_copy(xcb[:], xc[:])
            sqc = st_p.tile([128, DC, S], BF16, tag="sqc")
            nc.scalar.activation(sqc[:], xcb[:], AF.Square)
            fs_x = ps_sm.tile([1, S], F32, tag="sm")
            fs_q = ps_sm.tile([1, S], F32, tag="sm")
            for c in range(DC):
                nc.tensor.matmul(fs_x[:], ones1[:], xcb[:, c, :], start=(c == 0),
                                 stop=(c == DC - 1))
            for c in range(DC):
                nc.tensor.matmul(fs_q[:], ones1[:], sqc[:, c, :], start=(c == 0),
                                 stop=(c == DC - 1))
            fmean = st_p.tile([1, S], F32, tag="fmean")
            nc.vector.tensor_scalar_mul(fmean[:], fs_x[:], 1.0 / D)
            var = st_p.tile([1, S], F32, tag="fvar")
            nc.vector.tensor_scalar_mul(var[:], fs_q[:], 1.0 / D)
            fmsq = st_p.tile([1, S], F32, tag="fmsq")
            nc.vector.tensor_tensor(fmsq[:], fmean[:], fmean[:], ALU.mult)
            nc.vector.tensor_tensor(var[:], var[:], fmsq[:], ALU.subtract)
            r = st_p.tile([1, S], F32, tag="fr")
            nc.scalar.activation(r[:], var[:], AF.Ln, bias=EPS)
            nc.scalar.activation(r[:], r[:], AF.Exp, scale=-0.5)
            rb = st_p.tile([128, S], F32, tag="frb")
            nc.gpsimd.partition_broadcast(rb[:], r[:])
            mb = st_p.tile([128, S], F32, tag="fmb")
            nc.gpsimd.partition_broadcast(mb[:], fmean[:])
            on = st_p.tile([128, DC, S], F32, tag="on")
            for c in range(DC):
                nc.vector.tensor_tensor(on[:, c, :], xc[:, c, :], mb[:], ALU.subtract)
                nc.vector.tensor_tensor(on[:, c, :], on[:, c, :], rb[:], ALU.mult)
                nc.vector.tensor_scalar(on[:, c, :], on[:, c, :],
                                        wnorm_t[:, c, 0:1], wnorm_t[:, c, 1:2],
                                        ALU.mult, ALU.add)
            for c in range(DC):
                dst = bass.AP(tensor=out_d, offset=c * 128, ap=[[1, 128], [D, S]])
                nc.sync.dma_start(out=dst, in_=on[:, c, :])

    nc.compile()
    _CACHE[key] = nc
    return nc


def kernel(_n_layers=L, _dbg=False, **inputs):
    shared, per_core, flags = _prep(inputs)
    nc = _build(flags, n_layers=_n_layers, dbg=_dbg)
    in_maps = []
    for core in range(NCORES):
        m = dict(shared)
        m.update(per_core[core])
        in_maps.append(m)
    try:
        res = run_bass_kernel_spmd(nc, in_maps, core_ids=list(range(NCORES)))
    except Exception:
        # transient NRT faults have been observed once; one retry
        res = run_bass_kernel_spmd(nc, in_maps, core_ids=list(range(NCORES)))
    out = np.concatenate([res.results[i]["out"] for i in range(NCORES)], axis=0)
    if _dbg:
        xd = [res.results[i]["xdbg"] for i in range(NCORES)]
        return out.astype(np.float32), xd
    return out.astype(np.float32)



# revision 48
# speedup vs baseline: 1.6093x; 1.6093x over previous
"""ViT-Base + per-sample MoE adapters on 8 TRN2 NeuronCores.

Sharding: data-parallel over batch (4 samples/core, zero collectives).
Device layout: feature-major activations xT[d, t] (d on partitions, 6 chunks
of 128), bf16 matmul operands, fp32 residual. Scores are computed transposed
(sT[j,i]) so the softmax reduction becomes a ones-matmul and no on-chip
transposes are needed anywhere. LN gamma/beta and layer-scale are folded into
weights on the host; the adapter expert gather happens on the host during
sharding (it is per-sample indexing, i.e. data movement, not compute).
"""

import os
import sys

sys.path.insert(0, "/opt/trn_rl_repo")
sys.path.insert(0, "/root/.axon_site/_ro/trn_rl_repo")

from contextlib import ExitStack

import numpy as np
import ml_dtypes
from einops import rearrange

import concourse.bass as bass
import concourse.tile as tile
import concourse.mybir as mybir
from concourse import bacc
from concourse import bass_utils as _bu
from concourse.bass_utils import run_bass_kernel_spmd

# Restrict the ACT table sets to the two this kernel needs
# (natural_log_exp_and_others for LN rstd + softmax, gelu_and_others for MLP).
# With the full catalog, walrus bounces through extra sets on Square/Copy ops
# and the kernel pays ~139 table loads instead of ~26 (2.7us each, and they
# stall psum evictions). Graceful fallback to the stock file on any error.
def _setup_act_tables():
    try:
        import glob
        import json
        import tempfile
        from neuronxcc.driver.Job import Job

        cands = glob.glob(os.path.join(Job.getPackageDir(), "pwp",
                                       "pwp_bin_trainium*", "act_info.json"))
        if not cands:
            return
        src = cands[0]
        with open(src) as f:
            d = json.load(f)
        keep = {"natural_log_exp_and_others", "gelu_and_others"}
        d["act_func_sets"] = [s for s in d["act_func_sets"] if s["name"] in keep]
        if len(d["act_func_sets"]) != 2:
            return
        dstdir = tempfile.mkdtemp(prefix="act_custom_")
        import shutil

        srcdir = os.path.dirname(src)
        for fn in os.listdir(srcdir):
            if fn.endswith((".bin", ".json")) and fn != os.path.basename(src):
                try:
                    os.symlink(os.path.join(srcdir, fn), os.path.join(dstdir, fn))
                except OSError:
                    shutil.copy(os.path.join(srcdir, fn), os.path.join(dstdir, fn))
        dst = os.path.join(dstdir, os.path.basename(src))
        with open(dst, "w") as f:
            json.dump(d, f)
        os.environ["BASS_ACT_ROOT_JSON_PATH"] = dst
    except Exception:
        pass


_setup_act_tables()

# walrus's --enable-ldw-opt dedupes *consecutive identical* stationary-weight
# loads; the matmul loops below are ordered so each lhsT tile is used by two
# back-to-back matmuls (tb pairs), which halves LDWEIGHTS traffic there.
if not getattr(_bu.subprocess, "_ldwopt_patched", False):
    _orig_check_call = _bu.subprocess.check_call

    def _cc(argv, *a, **kw):
        if isinstance(argv, list) and argv and "walrus" in str(argv[0]):
            argv = ["--enable-ldw-opt=true" if x == "--enable-ldw-opt=false" else x
                    for x in argv]
        return _orig_check_call(argv, *a, **kw)

    _bu.subprocess.check_call = _cc
    _bu.subprocess._ldwopt_patched = True

# bass pre-places the table loads itself (bacc.insert_act_table_loads) using
# hw_specs.get_activation_tables; filter it to the same two sets so the
# pre-placed act_func_set_ids match the trimmed act_info.json walrus sees.
if "BASS_ACT_ROOT_JSON_PATH" in os.environ:
    from concourse import hw_specs as _hw

    _KEEP_SETS = ("natural_log_exp_and_others", "gelu_and_others")
    _orig_gat = _hw.get_activation_tables

    def _gat(arch):
        d = _orig_gat(arch)
        f = {k: d[k] for k in d if k in _KEEP_SETS}
        return f if len(f) == 2 else d

    _hw.get_activation_tables = _gat
    bacc.get_activation_tables = _gat

F32 = mybir.dt.float32
BF16 = mybir.dt.bfloat16
FP8 = mybir.dt.float8e4
AF = mybir.ActivationFunctionType
ALU = mybir.AluOpType
BF = ml_dtypes.bfloat16
# fp8 weights measured: per-layer quantization error is systematic (fixed
# weights), accumulates linearly over 12 layers -> 6e-2 rel err. Keep bf16.
FP8_MLP = False
TPAD = 800           # 16-byte-aligned token stride for fp8 pair layouts
DRMM = mybir.MatmulPerfMode.DoubleRow

B, IMG, PP, CIN = 32, 224, 16, 3
D, H, L, A, E, DD, FF = 768, 12, 12, 6, 8, 64, 3072
G = IMG // PP         # 14
N = G * G + 1         # 197
HD = D // H           # 64
NCORES = 8
S = B // NCORES       # 4 samples per core
T = S * N             # 788 tokens per core
DC = D // 128         # 6 chunks
QKC = 12              # q(6) + k(6) feature chunks
FJ = FF // 128        # 24
PC = (CIN * PP * PP) // 128  # 18
NPATCH = G * G        # 196
TB = 2
TBW = T // TB         # 394
EPS = 1e-6

_CACHE = {}


def _f(x):
    return np.asarray(x, np.float32)


def _prep(inputs):
    """Host-side prep: im2col, LN/LS folds, expert gather, bf16 packs."""
    pw = _f(inputs["patch_w"]); pb = _f(inputs["patch_b"])
    cls = _f(inputs["cls_token"]); pos = _f(inputs["pos_embed"])
    l1g = _f(inputs["ln1_g"]); l1b = _f(inputs["ln1_b"])
    qkvw = _f(inputs["qkv_w"]); qkvb = _f(inputs["qkv_b"])
    pjw = _f(inputs["proj_w"]); pjb = _f(inputs["proj_b"])
    ls1 = _f(inputs["ls1"]); ls2 = _f(inputs["ls2"])
    l2g = _f(inputs["ln2_g"]); l2b = _f(inputs["ln2_b"])
    f1w = _f(inputs["fc1_w"]); f1b = _f(inputs["fc1_b"])
    f2w = _f(inputs["fc2_w"]); f2b = _f(inputs["fc2_b"])
    ng = _f(inputs["norm_g"]); nb = _f(inputs["norm_b"])
    adw = _f(inputs["ad_down_w"]); adb = _f(inputs["ad_down_b"])
    auw = _f(inputs["ad_up_w"]); aub = _f(inputs["ad_up_b"])
    eids = np.asarray(inputs["expert_ids"], np.int64)
    imgs = _f(inputs["inputs"])

    shared = {}
    F8 = ml_dtypes.float8_e4m3
    qw = qkvw[:, :, :D]; kw = qkvw[:, :, D:2 * D]; vw = qkvw[:, :, 2 * D:]
    wqk = np.concatenate([qw, kw], axis=2) * l1g[:, :, None]          # [L,768,1536]
    # fp8 DoubleRow pair layout: partition pc carries contraction rows
    # (g*256+pc, g*256+128+pc); weights pre-scaled x32, activations x8,
    # descale 1/256 at psum evict. Quantization error is softmax/residual
    # damped here (measured 1.0e-2 final rel err in the numpy replica).
    shared["wqk8"] = rearrange(wqk * 32.0, "l (g o pc) (j pj) -> l j pc g o pj",
                               o=2, pc=128, pj=128).astype(F8)
    qkb = np.einsum("ldk,ld->lk", np.concatenate([qw, kw], axis=2), l1b) + qkvb[:, :2 * D]
    shared["qkb"] = rearrange(qkb, "l (j pj) -> l pj j", pj=128).astype(np.float32)

    wv = vw * l1g[:, :, None]
    shared["wv8"] = rearrange(wv * 32.0, "l (g o pc) d -> l pc g o d",
                              o=2, pc=128).astype(F8)
    vb = np.einsum("ldk,ld->lk", vw, l1b) + qkvb[:, 2 * D:]           # [L,768]

    wproj = pjw * ls1[:, None, :]
    shared["wproj8"] = rearrange(wproj * 32.0, "l (g o pc) (j pj) -> l j pc g o pj",
                                 o=2, pc=128, pj=128).astype(F8)
    pbe = ls1 * (pjb + np.einsum("ldk,ld->lk", pjw, vb))
    shared["pbe"] = rearrange(pbe, "l (j pj) -> l pj j", pj=128).astype(np.float32)

    wfc1 = f1w * l2g[:, :, None]
    shared["wfc1"] = rearrange(wfc1, "l (c pc) (j pj) -> l j pc c pj", pc=128, pj=128).astype(BF)
    f1be = np.einsum("ldk,ld->lk", f1w, l2b) + f1b
    shared["f1be"] = rearrange(f1be, "l (j pj) -> l pj j", pj=128).astype(np.float32)

    wfc2 = f2w * ls2[:, None, :]
    shared["wfc2"] = rearrange(wfc2, "l (c pc) (j pj) -> l j pc c pj", pc=128, pj=128).astype(BF)
    f2be = ls2 * f2b
    shared["f2be"] = rearrange(f2be, "l (j pj) -> l pj j", pj=128).astype(np.float32)

    wpatch = pw.T  # [2304, 768]
    shared["wpatch"] = rearrange(wpatch, "(c pc) (j pj) -> c pc j pj", pc=128, pj=128).astype(BF)

    posb = pos[0].copy()                  # [197, 768]
    posb[1:] += pb[None, :]
    posb[0] += cls[0, 0]
    shared["posbias"] = rearrange(posb, "t (c pc) -> pc c t", pc=128).astype(np.float32)

    shared["wnorm"] = np.stack([
        rearrange(ng, "(c pc) -> pc c", pc=128),
        rearrange(nb, "(c pc) -> pc c", pc=128)], axis=-1).astype(np.float32)  # [128,6,2]

    flags = dict(
        has_pbe=bool(np.abs(pbe).max() > 0),
        has_f2be=bool(np.abs(f2be).max() > 0),
        has_qkb=bool(np.abs(qkb).max() > 0),
        has_adb=bool(np.abs(adb).max() > 0),
        has_aub=bool(np.abs(aub).max() > 0),
    )

    per_core = []
    for core in range(NCORES):
        sl = slice(core * S, (core + 1) * S)
        im = imgs[sl]
        xp = im.reshape(S, CIN, G, PP, G, PP).transpose(0, 2, 4, 1, 3, 5).reshape(
            S * NPATCH, CIN * PP * PP)
        xpT = rearrange(np.ascontiguousarray(xp.T), "(c pc) t -> c pc t", pc=128).astype(BF)
        eid = eids[sl]
        pc_map = {
            "xpT": xpT,                                              # [18,128,784]
            "adwg": rearrange(adw[:, eid], "a s (c pc) k -> a pc c s k", pc=128).astype(BF),
            "auwg": rearrange(auw[:, eid], "a s k d -> a k s d").astype(BF),   # [A,64,S,768]
            "adbg": rearrange(adb[:, eid], "a s k -> a k s").astype(np.float32),
            "aubg": rearrange(aub[:, eid], "a s (j pj) -> a s pj j", pj=128).astype(np.float32),
        }
        per_core.append(pc_map)
    return shared, per_core, flags


def _build(flags, n_layers=L, dbg=False):
    key = (tuple(sorted(flags.items())), n_layers, dbg)
    if key in _CACHE:
        return _CACHE[key]
    nc = bacc.Bacc("TRN2", target_bir_lowering=False, debug=False, num_devices=NCORES)

    def din(name, shape, dt):
        return nc.dram_tensor(name, list(shape), dt, kind="ExternalInput").ap()

    xpT_d = din("xpT", [PC, 128, S * NPATCH], BF16)
    posb_d = din("posbias", [128, DC, N], F32)
    wqk_d = din("wqk8", [L, QKC, 128, DC // 2, 2, 128], FP8)
    qkb_d = din("qkb", [L, 128, QKC], F32)
    wv_d = din("wv8", [L, 128, DC // 2, 2, D], FP8)
    wproj_d = din("wproj8", [L, DC, 128, DC // 2, 2, 128], FP8)
    pbe_d = din("pbe", [L, 128, DC], F32)
    wfc1_d = din("wfc1", [L, FJ, 128, DC, 128], BF16)
    f1be_d = din("f1be", [L, 128, FJ], F32)
    wfc2_d = din("wfc2", [L, DC, 128, FJ, 128], BF16)
    f2be_d = din("f2be", [L, 128, DC], F32)
    wpatch_d = din("wpatch", [PC, 128, DC, 128], BF16)
    wnorm_d = din("wnorm", [128, DC, 2], F32)
    adwg_d = din("adwg", [A, 128, DC, S, DD], BF16)
    auwg_d = din("auwg", [A, DD, S, D], BF16)
    adbg_d = din("adbg", [A, DD, S], F32)
    aubg_d = din("aubg", [A, S, 128, DC], F32)

    out_d = nc.dram_tensor("out", [S, D], F32, kind="ExternalOutput")
    if dbg:
        xdbg_d = nc.dram_tensor("xdbg", [128, DC, T], F32, kind="ExternalOutput").ap()

    scol = [slice(s * N, (s + 1) * N) for s in range(S)]
    tbcol = [slice(tb * TBW, (tb + 1) * TBW) for tb in range(TB)]
    jts = [(0, 128), (128, N)]

    with tile.TileContext(nc) as tc:
        with ExitStack() as ctx:
            per = ctx.enter_context(tc.tile_pool(name="per", bufs=1))
            wq_p = ctx.enter_context(tc.tile_pool(name="wq", bufs=3))
            wf2_p = ctx.enter_context(tc.tile_pool(name="wf2", bufs=2))
            xp_p = ctx.enter_context(tc.tile_pool(name="xp", bufs=2))
            ad_p = ctx.enter_context(tc.tile_pool(name="ad", bufs=1))
            bia_p = ctx.enter_context(tc.tile_pool(name="bia", bufs=2))
            st_p = ctx.enter_context(tc.tile_pool(name="st", bufs=3))
            stp4 = ctx.enter_context(tc.tile_pool(name="stp4", bufs=4))
            exp_p = ctx.enter_context(tc.tile_pool(name="exp", bufs=4))
            lno_p = ctx.enter_context(tc.tile_pool(name="lno", bufs=2))
            sq_p = ctx.enter_context(tc.tile_pool(name="sq", bufs=1))
            xbf_p = ctx.enter_context(tc.tile_pool(name="xbf", bufs=2))
            ps_mm = ctx.enter_context(tc.tile_pool(name="psmm", bufs=3, space="PSUM"))
            ps_at = ctx.enter_context(tc.tile_pool(name="psat", bufs=3, space="PSUM"))
            ps_sm = ctx.enter_context(tc.tile_pool(name="pssm", bufs=2, space="PSUM"))

            x = per.tile([128, DC, T], F32, tag="x")
            qk = per.tile([128, QKC, T], BF16, tag="qk")
            v_tok = per.tile([128, S, 2, D], BF16, tag="vtok")
            attn = per.tile([128, DC // 2, 2, TPAD], FP8, tag="attn")
            hml = per.tile([128, FJ, T], BF16, tag="hml")
            wv_t = per.tile([128, DC // 2, 2, D], FP8, tag="wvt")
            posb = per.tile([128, DC, N], F32, tag="posb")
            ones1 = per.tile([128, 1], BF16, tag="ones1")
            ones8 = per.tile([128, 1], BF16, tag="ones8")
            wnorm_t = per.tile([128, DC, 2], F32, tag="wnormt")

            nc.vector.memset(ones1[:], 1.0)
            nc.vector.memset(ones8[:], 0.125)
            czero = per.tile([128, 1], F32, tag="czero")
            nc.vector.memset(czero[:], 0.0)
            ceps = per.tile([128, 1], F32, tag="ceps")
            nc.vector.memset(ceps[:], EPS)
            clog8 = per.tile([128, 1], F32, tag="clog8")
            nc.vector.memset(clog8[:], float(np.log(8.0)))
            nc.const_aps.aps[(F32, 0.0)] = czero[:]
            nc.const_aps.aps[(F32, EPS)] = ceps[:]
            nc.const_aps.aps[(F32, float(np.log(8.0)))] = clog8[:]
            nc.sync.dma_start(out=posb[:], in_=posb_d[:])
            nc.sync.dma_start(out=wnorm_t[:], in_=wnorm_d[:])

            # ======== patch embed ========
            # c-outer / j-inner: each xp chunk is DMA'd once per sample-half
            # and feeds all 6 output chunks held in 6 live psum banks
            # (3 from ps_mm + 3 from ps_at; nothing else uses psum yet).
            x_bf = xbf_p.tile([128, DC, T], BF16, tag="xbf")
            for sh in range(2):
                ps6 = []
                for j in range(DC):
                    psj = (ps_mm if j < 3 else ps_at).tile(
                        [128, 2 * NPATCH], F32, tag=("mm" if j < 3 else "at"),
                        name=f"pspe{j}")
                    ps6.append(psj)
                for c in range(PC):
                    wpc = wq_p.tile([128, DC, 128], BF16, tag="wq")
                    nc.sync.dma_start(out=wpc[:], in_=wpatch_d[c])
                    xpc = xp_p.tile([128, 2 * NPATCH], BF16, tag="xp")
                    nc.sync.dma_start(
                        out=xpc[:],
                        in_=xpT_d[c, :, sh * 2 * NPATCH:(sh + 1) * 2 * NPATCH])
                    for j in range(DC):
                        nc.tensor.matmul(ps6[j][:], wpc[:, j, :], xpc[:],
                                         start=(c == 0), stop=(c == PC - 1))
                for j in range(DC):
                    for si in range(2):
                        s = sh * 2 + si
                        nc.vector.tensor_tensor(
                            x[:, j, s * N + 1:(s + 1) * N],
                            ps6[j][:, si * NPATCH:(si + 1) * NPATCH],
                            posb[:, j, 1:N], ALU.add)
                        nc.vector.tensor_copy(x_bf[:, j, s * N + 1:(s + 1) * N],
                                              x[:, j, s * N + 1:(s + 1) * N])
            for j in range(DC):
                for s in range(S):
                    nc.vector.tensor_copy(x[:, j, s * N:s * N + 1], posb[:, j, 0:1])
                    nc.vector.tensor_copy(x_bf[:, j, s * N:s * N + 1], posb[:, j, 0:1])

            LOG8 = float(np.log(8.0))

            def layernorm_stats_tb(src_bf, sqt, tb, scale8=False):
                """One token-block's LN stats -> broadcast ab tile [128,2,TBW].
                Emitted at the tail of the producing matmul stage (proj/fc2)
                so the rstd scalar/vector/gpsimd chain overlaps the other
                token-block's PE work. scale8 folds the x8 fp8 headroom
                factor into ab (ln1 is stored fp8 at 8x scale)."""
                for c in range(DC):
                    nc.scalar.activation(sqt[:, c, :], src_bf[:, c, tbcol[tb]], AF.Square)
                sm_x = ps_sm.tile([1, TBW], F32, tag="sm")
                sm_q = ps_sm.tile([1, TBW], F32, tag="sm")
                for c in range(DC):
                    nc.tensor.matmul(sm_x[:], ones1[:], src_bf[:, c, tbcol[tb]],
                                     start=(c == 0), stop=(c == DC - 1))
                for c in range(DC):
                    nc.tensor.matmul(sm_q[:], ones1[:], sqt[:, c, :],
                                     start=(c == 0), stop=(c == DC - 1))
                # mA = mean; varD = Sum(x^2) - Sum(x)^2/D; r = (varD/D + eps)^-1/2
                mA = stp4.tile([1, TBW], F32, tag="stat")
                nc.vector.tensor_scalar_mul(mA[:], sm_x[:], 1.0 / D)
                msqD = stp4.tile([1, TBW], F32, tag="stat")
                nc.vector.tensor_tensor(msqD[:], mA[:], sm_x[:], ALU.mult)
                varD = stp4.tile([1, TBW], F32, tag="stat")
                nc.vector.tensor_tensor(varD[:], sm_q[:], msqD[:], ALU.subtract)
                r = stp4.tile([1, TBW], F32, tag="stat")
                nc.scalar.activation(r[:], varD[:], AF.Ln, bias=EPS, scale=1.0 / D)
                ab = st_p.tile([1, 2, TBW], BF16, tag="ab")
                nc.scalar.activation(ab[:, 0, :], r[:], AF.Exp, scale=-0.5,
                                     bias=LOG8 if scale8 else 0.0)
                mr = stp4.tile([1, TBW], F32, tag="stat")
                nc.vector.tensor_tensor(mr[:], mA[:], ab[:, 0, :], ALU.mult)
                nc.vector.tensor_scalar_mul(ab[:, 1, :], mr[:], -1.0)
                abb = st_p.tile([128, 2, TBW], BF16, tag="abb")
                nc.gpsimd.partition_broadcast(abb[:], ab[0:1, :, :])
                return abb

            def ln_apply_tb(src_bf, abb, dst, tb, f8=False):
                for c in range(DC):
                    if f8:
                        tmp = stp4.tile([128, TBW], BF16, tag="lntmp", bufs=2)
                        nc.vector.tensor_tensor(tmp[:], src_bf[:, c, tbcol[tb]],
                                                abb[:, 0, :], ALU.mult)
                        nc.vector.tensor_tensor(dst[:, c // 2, c % 2, tbcol[tb]],
                                                tmp[:], abb[:, 1, :], ALU.add)
                    else:
                        nc.vector.tensor_tensor(dst[:, c, tbcol[tb]],
                                                src_bf[:, c, tbcol[tb]],
                                                abb[:, 0, :], ALU.mult)
                        nc.vector.tensor_tensor(dst[:, c, tbcol[tb]],
                                                dst[:, c, tbcol[tb]],
                                                abb[:, 1, :], ALU.add)

            # ======== transformer layers ========
            ln1_t = None
            for l in range(n_layers):
                # ---- LN1 ---- (x_bf shadow-written by patch/fc2 evicts;
                # stats+apply for l>0 were emitted at the tail of fc2)
                if ln1_t is None:
                    sqt = sq_p.tile([128, DC, TBW], BF16, tag="sq", name="sqt0")
                    ln1 = lno_p.tile([128, DC // 2, 2, TPAD], FP8, tag="lnof8",
                                     name="ln1_0", bufs=1)
                    for tb in range(TB):
                        abb0 = layernorm_stats_tb(x_bf, sqt, tb, scale8=True)
                        ln_apply_tb(x_bf, abb0, ln1, tb, f8=True)
                else:
                    ln1 = ln1_t

                # ---- QK ---- staggered: tb0 chains run 2 j ahead of tb1 so
                # the PE never sits behind the LN1(tb1) apply; q/k chunk pairs
                # emitted together so attention head-pair a can start early.
                if flags["has_qkb"]:
                    qkb_t = bia_p.tile([128, QKC], F32, tag="qkb")
                    nc.sync.dma_start(out=qkb_t[:], in_=qkb_d[l])

                def qk_evict(j, tb, ps):
                    # descale the x8-act x32-weight fp8 product
                    if flags["has_qkb"]:
                        nc.scalar.activation(qk[:, j, tbcol[tb]], ps[:], AF.Identity,
                                             scale=1.0 / 256, bias=qkb_t[:, j:j + 1])
                    else:
                        nc.scalar.mul(qk[:, j, tbcol[tb]], ps[:], 1.0 / 256)

                for j in [0, 6, 1, 7, 2, 8, 3, 9, 4, 10, 5, 11]:
                    wj = wq_p.tile([128, DC // 2, 2, 128], FP8, tag="wq")
                    nc.sync.dma_start(out=wj[:], in_=wqk_d[l, j])
                    ps0 = ps_mm.tile([128, TBW], F32, tag="mm")
                    ps1 = ps_mm.tile([128, TBW], F32, tag="mm")
                    for g in range(DC // 2):  # same lhsT twice -> ldw-opt dedupes
                        nc.tensor.matmul(ps0[:], wj[:, g, :, :],
                                         ln1[:, g, :, tbcol[0]],
                                         start=(g == 0), stop=(g == DC // 2 - 1),
                                         perf_mode=DRMM)
                        nc.tensor.matmul(ps1[:], wj[:, g, :, :],
                                         ln1[:, g, :, tbcol[1]],
                                         start=(g == 0), stop=(g == DC // 2 - 1),
                                         perf_mode=DRMM)
                    qk_evict(j, 0, ps0)
                    qk_evict(j, 1, ps1)

                # ---- V (token-major) ----
                nc.sync.dma_start(out=wv_t[:], in_=wv_d[l])
                for s in range(S):
                    for jt, (j0, j1) in enumerate(jts):
                        tn = j1 - j0
                        ps0 = ps_mm.tile([128, 384], F32, tag="mm")
                        ps1 = ps_mm.tile([128, 384], F32, tag="mm")
                        for c in range(DC):
                            lh = ln1[:, c // 2, c % 2, s * N + j0:s * N + j1]
                            nc.tensor.matmul(ps0[:tn, :], lh,
                                             wv_t[:, c // 2, c % 2, 0:384],
                                             start=(c == 0), stop=(c == DC - 1))
                            nc.tensor.matmul(ps1[:tn, :], lh,
                                             wv_t[:, c // 2, c % 2, 384:768],
                                             start=(c == 0), stop=(c == DC - 1))
                        nc.vector.tensor_scalar_mul(v_tok[:tn, s, jt, 0:384],
                                                    ps0[:tn, :], 1.0 / 256)
                        nc.vector.tensor_scalar_mul(v_tok[:tn, s, jt, 384:768],
                                                    ps1[:tn, :], 1.0 / 256)

                # ---- attention ---- software-pipelined over (s, head-pair).
                # Stages: st0 scores+exp -> st1 ones-sum+recip+bcast -> st2
                # attnV -> st3 evict. Lookahead-2 emission keeps independent
                # score streams in front of the PE queue while the current
                # item's exp/recip/broadcast chain resolves on the other
                # engines.
                items = [(s, a) for s in range(S) for a in range(DC)]
                at_exp, at_rcb, at_ps = {}, {}, {}

                def att_st0(it):
                    s, a = it
                    sT_e = ps_at.tile([128, 2, N], F32, tag="at")
                    sT_o = ps_at.tile([128, 2, N], F32, tag="at")
                    # jt0+jt1 chained into one accumulation group per psum
                    # tile (disjoint halves, start=True on both) -> one
                    # group-end drain instead of two.
                    for sT, p0, tp in ((sT_e, 0, (0, 0)), (sT_o, 64, (64, 0))):
                        for jt, (j0, j1) in enumerate(jts):
                            tn = j1 - j0
                            nc.tensor.matmul(
                                sT[:tn, jt, :],
                                qk[p0:p0 + 64, DC + a, scol[s]][:, j0:j1],
                                qk[p0:p0 + 64, a, scol[s]],
                                start=True, stop=(jt == 1),
                                skip_group_check=(jt == 1),
                                tile_position=tp)
                    expe = exp_p.tile([128, 2, N], BF16, tag="exp")
                    expo = exp_p.tile([128, 2, N], BF16, tag="exp")
                    # one wide exp per head; rows 69-127 of the jt=1 slice are
                    # garbage (never read by the cs/oT matmuls below)
                    nc.scalar.activation(expe[:], sT_e[:], AF.Exp, scale=0.125)
                    nc.scalar.activation(expo[:], sT_o[:], AF.Exp, scale=0.125)
                    at_exp[it] = (expe, expo)

                def att_st1(it):
                    expe, expo = at_exp[it]
                    cs = ps_sm.tile([1, 2, N], F32, tag="sm")
                    nc.tensor.matmul(cs[:, 0, :], ones8[:], expe[:, 0, :],
                                     start=True, stop=False)
                    nc.tensor.matmul(cs[:, 0, :], ones8[:69, :], expe[:69, 1, :],
                                     start=False, stop=False)
                    nc.tensor.matmul(cs[:, 1, :], ones8[:], expo[:, 0, :],
                                     start=True, stop=False, skip_group_check=True)
                    nc.tensor.matmul(cs[:, 1, :], ones8[:69, :], expo[:69, 1, :],
                                     start=False, stop=True)
                    rec = st_p.tile([1, 2, N], F32, tag="rec", bufs=2)
                    nc.vector.reciprocal_approx_fast(rec[:], cs[:])
                    rcb = st_p.tile([128, 2, N], F32, tag="rcb", bufs=2)
                    nc.gpsimd.partition_broadcast(rcb[:], rec[0:1, :, :])
                    at_rcb[it] = rcb

                def att_st2(it):
                    s, a = it
                    expe, expo = at_exp[it]
                    psA = ps_mm.tile([128, N], F32, tag="mm")
                    psB = ps_mm.tile([128, N], F32, tag="mm")
                    dlo = a * 128
                    nc.tensor.matmul(psA[:], v_tok[:, s, 0, dlo:dlo + 128], expe[:, 0, :],
                                     start=True, stop=False)
                    nc.tensor.matmul(psB[:], v_tok[:, s, 0, dlo:dlo + 128], expo[:, 0, :],
                                     start=True, stop=False)
                    nc.tensor.matmul(psA[:], v_tok[:69, s, 1, dlo:dlo + 128],
                                     expe[:69, 1, :], start=False, stop=True)
                    nc.tensor.matmul(psB[:], v_tok[:69, s, 1, dlo:dlo + 128],
                                     expo[:69, 1, :], start=False, stop=True)
                    at_ps[it] = (psA, psB)

                def att_st3(it):
                    s, a = it
                    psA, psB = at_ps.pop(it)
                    rcb = at_rcb.pop(it)
                    at_exp.pop(it)
                    nc.vector.tensor_tensor(attn[0:64, a // 2, a % 2, scol[s]],
                                            psA[0:64, :], rcb[0:64, 0, :], ALU.mult)
                    nc.vector.tensor_tensor(attn[64:128, a // 2, a % 2, scol[s]],
                                            psB[64:128, :], rcb[64:128, 1, :], ALU.mult)

                att_st0(items[0])
                att_st0(items[1])
                att_st1(items[0])
                for i in range(len(items)):
                    if i + 2 < len(items):
                        att_st0(items[i + 2])
                    if i + 1 < len(items):
                        att_st1(items[i + 1])
                    att_st2(items[i])
                    att_st3(items[i])

                # Last layer: only the CLS columns survive to the final LN, so
                # proj/LN2/MLP run on 4 columns instead of 788.
                last = (l == n_layers - 1) and (not dbg) and l >= A
                if last:
                    if flags["has_pbe"]:
                        pbe_t = bia_p.tile([128, DC], F32, tag="pbe")
                        nc.sync.dma_start(out=pbe_t[:], in_=pbe_d[l])
                    for j in range(DC):
                        wj = wq_p.tile([128, DC // 2, 2, 128], FP8, tag="wq")
                        nc.sync.dma_start(out=wj[:], in_=wproj_d[l, j])
                        ps = ps_mm.tile([128, S], F32, tag="mm")
                        for g in range(DC // 2):
                            nc.tensor.matmul(ps[:], wj[:, g, :, :],
                                             attn[:, g, :, 0:T:N],
                                             start=(g == 0), stop=(g == DC // 2 - 1),
                                             perf_mode=DRMM)
                        nc.vector.scalar_tensor_tensor(
                            x[:, j, ::N], ps[:], 1.0 / 256,
                            x[:, j, ::N], ALU.mult, ALU.add)
                        if flags["has_pbe"]:
                            nc.vector.tensor_scalar_add(x[:, j, ::N], x[:, j, ::N],
                                                        pbe_t[:, j:j + 1])
                    # LN2 on CLS columns
                    xbfc = st_p.tile([128, DC, S], BF16, tag="xbfc")
                    for c in range(DC):
                        nc.vector.tensor_copy(xbfc[:, c, :], x[:, c, ::N])
                    sqc2 = st_p.tile([128, DC, S], BF16, tag="sqc2")
                    nc.scalar.activation(sqc2[:], xbfc[:], AF.Square)
                    smc_x = ps_sm.tile([1, S], F32, tag="sm")
                    smc_q = ps_sm.tile([1, S], F32, tag="sm")
                    for c in range(DC):
                        nc.tensor.matmul(smc_x[:], ones1[:], xbfc[:, c, :],
                                         start=(c == 0), stop=(c == DC - 1))
                    for c in range(DC):
                        nc.tensor.matmul(smc_q[:], ones1[:], sqc2[:, c, :],
                                         start=(c == 0), stop=(c == DC - 1))
                    mAc = st_p.tile([1, S], F32, tag="mAc")
                    nc.vector.tensor_scalar_mul(mAc[:], smc_x[:], 1.0 / D)
                    msqc = st_p.tile([1, S], F32, tag="msqc")
                    nc.vector.tensor_tensor(msqc[:], mAc[:], smc_x[:], ALU.mult)
                    varc = st_p.tile([1, S], F32, tag="varc")
                    nc.vector.tensor_tensor(varc[:], smc_q[:], msqc[:], ALU.subtract)
                    rc = st_p.tile([1, S], F32, tag="rcl")
                    nc.scalar.activation(rc[:], varc[:], AF.Ln, bias=EPS, scale=1.0 / D)
                    abc = st_p.tile([1, 2, S], BF16, tag="abc")
                    nc.scalar.activation(abc[:, 0, :], rc[:], AF.Exp, scale=-0.5)
                    mrc = st_p.tile([1, S], F32, tag="mrc")
                    nc.vector.tensor_tensor(mrc[:], mAc[:], abc[:, 0, :], ALU.mult)
                    nc.vector.tensor_scalar_mul(abc[:, 1, :], mrc[:], -1.0)
                    abbc = st_p.tile([128, 2, S], BF16, tag="abbc")
                    nc.gpsimd.partition_broadcast(abbc[:], abc[0:1, :, :])
                    ln2c = st_p.tile([128, DC, S], BF16, tag="ln2c")
                    for c in range(DC):
                        nc.vector.tensor_tensor(ln2c[:, c, :], xbfc[:, c, :],
                                                abbc[:, 0, :], ALU.mult)
                        nc.vector.tensor_tensor(ln2c[:, c, :], ln2c[:, c, :],
                                                abbc[:, 1, :], ALU.add)
                    # MLP on CLS columns
                    f1be_t = bia_p.tile([128, FJ], F32, tag="f1b")
                    nc.sync.dma_start(out=f1be_t[:], in_=f1be_d[l])
                    if flags["has_f2be"]:
                        f2be_t = bia_p.tile([128, DC], F32, tag="f2b")
                        nc.sync.dma_start(out=f2be_t[:], in_=f2be_d[l])
                    hc = st_p.tile([128, FJ, S], BF16, tag="hc")
                    for j in range(FJ):
                        wj = wq_p.tile([128, DC, 128], BF16, tag="wq")
                        nc.sync.dma_start(out=wj[:], in_=wfc1_d[l, j])
                        ps = ps_mm.tile([128, S], F32, tag="mm")
                        for c in range(DC):
                            nc.tensor.matmul(ps[:], wj[:, c, :], ln2c[:, c, :],
                                             start=(c == 0), stop=(c == DC - 1))
                        nc.scalar.activation(hc[:, j, :], ps[:], AF.Gelu,
                                             bias=f1be_t[:, j:j + 1])
                    for j in range(DC):
                        w2j = wf2_p.tile([128, FJ, 128], BF16, tag="wf2")
                        nc.sync.dma_start(out=w2j[:], in_=wfc2_d[l, j])
                        ps = ps_mm.tile([128, S], F32, tag="mm")
                        for c in range(FJ):
                            nc.tensor.matmul(ps[:], w2j[:, c, :], hc[:, c, :],
                                             start=(c == 0), stop=(c == FJ - 1))
                        if flags["has_f2be"]:
                            nc.vector.scalar_tensor_tensor(
                                x[:, j, ::N], ps[:], f2be_t[:, j:j + 1],
                                x[:, j, ::N], ALU.add, ALU.add)
                        else:
                            nc.vector.tensor_tensor(x[:, j, ::N], ps[:],
                                                    x[:, j, ::N], ALU.add)
                    continue

                # ---- proj + residual (shadow bf16 for LN2/adapter) ----
                x_bf2 = xbf_p.tile([128, DC, T], BF16, tag="xbf")
                if flags["has_pbe"]:
                    pbe_t = bia_p.tile([128, DC], F32, tag="pbe")
                    nc.sync.dma_start(out=pbe_t[:], in_=pbe_d[l])
                # tb-outer so LN2 stats(tb0) + rstd + apply(tb0) overlap
                # proj(tb1) PE work (costs a second wproj DMA pass, 1.2MB).
                sqt2 = sq_p.tile([128, DC, TBW], BF16, tag="sq")
                ln2 = lno_p.tile([128, DC, T], BF16, tag="lno")
                for tb in range(TB):
                    for j in range(DC):
                        wj = wq_p.tile([128, DC // 2, 2, 128], FP8, tag="wq")
                        nc.sync.dma_start(out=wj[:], in_=wproj_d[l, j])
                        ps = ps_mm.tile([128, TBW], F32, tag="mm")
                        for g in range(DC // 2):
                            nc.tensor.matmul(ps[:], wj[:, g, :, :],
                                             attn[:, g, :, tbcol[tb]],
                                             start=(g == 0), stop=(g == DC // 2 - 1),
                                             perf_mode=DRMM)
                        nc.vector.scalar_tensor_tensor(
                            x[:, j, tbcol[tb]], ps[:], 1.0 / 256,
                            x[:, j, tbcol[tb]], ALU.mult, ALU.add)
                        if flags["has_pbe"]:
                            nc.vector.tensor_scalar_add(
                                x[:, j, tbcol[tb]], x[:, j, tbcol[tb]],
                                pbe_t[:, j:j + 1])
                        nc.vector.tensor_copy(x_bf2[:, j, tbcol[tb]],
                                              x[:, j, tbcol[tb]])
                    abb2 = layernorm_stats_tb(x_bf2, sqt2, tb)
                    ln_apply_tb(x_bf2, abb2, ln2, tb)

                # ---- adapter ----
                if l < A:
                    adw_t = ad_p.tile([128, DC, S, DD], BF16, tag="adw")
                    auw_t = ad_p.tile([DD, S, D], BF16, tag="auw")
                    nc.sync.dma_start(out=adw_t[:], in_=adwg_d[l])
                    nc.sync.dma_start(out=auw_t[:], in_=auwg_d[l])
                    if flags["has_adb"]:
                        adbg_t = bia_p.tile([DD, S], F32, tag="adb")
                        nc.sync.dma_start(out=adbg_t[:], in_=adbg_d[l])
                    if flags["has_aub"]:
                        aubg_t = bia_p.tile([S, 128, DC], F32, tag="aub")
                        nc.sync.dma_start(out=aubg_t[:], in_=aubg_d[l])
                def adapter_compute(adw_t=None, auw_t=None, x_src=None,
                                    adbg=None, aubg=None):
                    # psums come from ps_at (idle during the MLP); gelus are
                    # emitted mid-fc1 so they join the gelu table-set run
                    # instead of thrashing against LN exp/ln loads.
                    for s in range(S):
                        psh = ps_at.tile([DD, N], F32, tag="at")
                        for c in range(DC):
                            nc.tensor.matmul(psh[:], adw_t[:, c, s, :], x_src[:, c, scol[s]],
                                             start=(c == 0), stop=(c == DC - 1))
                        hp = st_p.tile([DD, N], BF16, tag="hp")
                        if adbg is not None:
                            nc.scalar.activation(hp[:], psh[:], AF.Gelu,
                                                 bias=adbg[:, s:s + 1])
                        else:
                            nc.scalar.activation(hp[:], psh[:], AF.Gelu)
                        for j in range(DC):
                            psu = ps_at.tile([128, N], F32, tag="at")
                            nc.tensor.matmul(psu[:], auw_t[:, s, j * 128:(j + 1) * 128],
                                             hp[:], start=True, stop=True)
                            if aubg is not None:
                                nc.vector.scalar_tensor_tensor(
                                    x[:, j, scol[s]], psu[:], aubg[s, :, j:j + 1],
                                    x[:, j, scol[s]], ALU.add, ALU.add)
                            else:
                                nc.vector.tensor_tensor(x[:, j, scol[s]], psu[:],
                                                        x[:, j, scol[s]], ALU.add)

                if l < A:
                    ad_args = dict(adw_t=adw_t, auw_t=auw_t, x_src=x_bf2,
                                   adbg=adbg_t if flags["has_adb"] else None,
                                   aubg=aubg_t if flags["has_aub"] else None)

                # ---- MLP ---- (fc2 evicts shadow next layer's LN1 input)
                x_bf = xbf_p.tile([128, DC, T], BF16, tag="xbf")
                f1be_t = bia_p.tile([128, FJ], F32, tag="f1b")
                nc.sync.dma_start(out=f1be_t[:], in_=f1be_d[l])
                if flags["has_f2be"]:
                    f2be_t = bia_p.tile([128, DC], F32, tag="f2b")
                    nc.sync.dma_start(out=f2be_t[:], in_=f2be_d[l])
                for j in range(FJ):
                    wj = wq_p.tile([128, DC, 128], BF16, tag="wq")
                    nc.sync.dma_start(out=wj[:], in_=wfc1_d[l, j])
                    ps0 = ps_mm.tile([128, TBW], F32, tag="mm")
                    ps1 = ps_mm.tile([128, TBW], F32, tag="mm")
                    for c in range(DC):
                        nc.tensor.matmul(ps0[:], wj[:, c, :], ln2[:, c, tbcol[0]],
                                         start=(c == 0), stop=(c == DC - 1))
                        nc.tensor.matmul(ps1[:], wj[:, c, :], ln2[:, c, tbcol[1]],
                                         start=(c == 0), stop=(c == DC - 1))
                    nc.scalar.activation(hml[:, j, tbcol[0]], ps0[:], AF.Gelu,
                                         bias=f1be_t[:, j:j + 1])
                    nc.scalar.activation(hml[:, j, tbcol[1]], ps1[:], AF.Gelu,
                                         bias=f1be_t[:, j:j + 1])
                    if j == 5 and l < A:
                        adapter_compute(**ad_args)
                # fc2 tb-outer: LN1(l+1) stats+apply for each tb run right
                # after its last evict; the rstd chain overlaps the other
                # tb's (or QK's) PE work.
                sqt_n = sq_p.tile([128, DC, TBW], BF16, tag="sq", name=f"sqtn{l}")
                ln1_t = lno_p.tile([128, DC // 2, 2, TPAD], FP8, tag="lnof8",
                                   name=f"ln1n{l}", bufs=1)
                for tb in range(TB):
                    for j in range(DC):
                        w2j = wf2_p.tile([128, FJ, 128], BF16, tag="wf2",
                                         name=f"w2j{tb}_{j}")
                        nc.sync.dma_start(out=w2j[:], in_=wfc2_d[l, j])
                        ps = ps_mm.tile([128, TBW], F32, tag="mm")
                        for c in range(FJ):
                            nc.tensor.matmul(ps[:], w2j[:, c, :], hml[:, c, tbcol[tb]],
                                             start=(c == 0), stop=(c == FJ - 1))
                        if flags["has_f2be"]:
                            nc.vector.scalar_tensor_tensor(
                                x[:, j, tbcol[tb]], ps[:], f2be_t[:, j:j + 1],
                                x[:, j, tbcol[tb]], ALU.add, ALU.add)
                        else:
                            nc.vector.tensor_tensor(x[:, j, tbcol[tb]], ps[:],
                                                    x[:, j, tbcol[tb]], ALU.add)
                        nc.vector.tensor_copy(x_bf[:, j, tbcol[tb]],
                                              x[:, j, tbcol[tb]])
                    abbn = layernorm_stats_tb(x_bf, sqt_n, tb, scale8=True)
                    ln_apply_tb(x_bf, abbn, ln1_t, tb, f8=True)

            if dbg:
                for c in range(DC):
                    nc.sync.dma_start(out=xdbg_d[:, c, :], in_=x[:, c, :])

            # ======== final LN on CLS columns + output ========
            xc = st_p.tile([128, DC, S], F32, tag="xc")
            for c in range(DC):
                nc.vector.tensor_copy(xc[:, c, :], x[:, c, ::N])
            xcb = st_p.tile([128, DC, S], BF16, tag="xcb")
            nc.vector.tensor_copy(xcb[:], xc[:])
            sqc = st_p.tile([128, DC, S], BF16, tag="sqc")
            nc.scalar.activation(sqc[:], xcb[:], AF.Square)
            fs_x = ps_sm.tile([1, S], F32, tag="sm")
            fs_q = ps_sm.tile([1, S], F32, tag="sm")
            for c in range(DC):
                nc.tensor.matmul(fs_x[:], ones1[:], xcb[:, c, :], start=(c == 0),
                                 stop=(c == DC - 1))
            for c in range(DC):
                nc.tensor.matmul(fs_q[:], ones1[:], sqc[:, c, :], start=(c == 0),
                                 stop=(c == DC - 1))
            fmean = st_p.tile([1, S], F32, tag="fmean")
            nc.vector.tensor_scalar_mul(fmean[:], fs_x[:], 1.0 / D)
            var = st_p.tile([1, S], F32, tag="fvar")
            nc.vector.tensor_scalar_mul(var[:], fs_q[:], 1.0 / D)
            fmsq = st_p.tile([1, S], F32, tag="fmsq")
            nc.vector.tensor_tensor(fmsq[:], fmean[:], fmean[:], ALU.mult)
            nc.vector.tensor_tensor(var[:], var[:], fmsq[:], ALU.subtract)
            r = st_p.tile([1, S], F32, tag="fr")
            nc.scalar.activation(r[:], var[:], AF.Ln, bias=EPS)
            nc.scalar.activation(r[:], r[:], AF.Exp, scale=-0.5)
            rb = st_p.tile([128, S], F32, tag="frb")
            nc.gpsimd.partition_broadcast(rb[:], r[:])
            mb = st_p.tile([128, S], F32, tag="fmb")
            nc.gpsimd.partition_broadcast(mb[:], fmean[:])
            on = st_p.tile([128, DC, S], F32, tag="on")
            for c in range(DC):
                nc.vector.tensor_tensor(on[:, c, :], xc[:, c, :], mb[:], ALU.subtract)
                nc.vector.tensor_tensor(on[:, c, :], on[:, c, :], rb[:], ALU.mult)
                nc.vector.tensor_scalar(on[:, c, :], on[:, c, :],
                                        wnorm_t[:, c, 0:1], wnorm_t[:, c, 1:2],
                                        ALU.mult, ALU.add)
            for c in range(DC):
                dst = bass.AP(tensor=out_d, offset=c * 128, ap=[[1, 128], [D, S]])
                nc.sync.dma_start(out=dst, in_=on[:, c, :])

    nc.compile()
    _CACHE[key] = nc
    return nc


def kernel(_n_layers=L, _dbg=False, **inputs):
    shared, per_core, flags = _prep(inputs)
    nc = _build(flags, n_layers=_n_layers, dbg=_dbg)
    in_maps = []
    for core in range(NCORES):
        m = dict(shared)
        m.update(per_core[core])
        in_maps.append(m)
    try:
        res = run_bass_kernel_spmd(nc, in_maps, core_ids=list(range(NCORES)))
    except Exception:
        # transient NRT faults have been observed once; one retry
        res = run_bass_kernel_spmd(nc, in_maps, core_ids=list(range(NCORES)))
    out = np.concatenate([res.results[i]["out"] for i in range(NCORES)], axis=0)
    if _dbg:
        xd = [res.results[i]["xdbg"] for i in range(NCORES)]
        return out.astype(np.float32), xd
    return out.astype(np.float32)



# revision 53
# speedup vs baseline: 1.6681x; 1.0365x over previous
"""ViT-Base + per-sample MoE adapters on 8 TRN2 NeuronCores.

Sharding: data-parallel over batch (4 samples/core, zero collectives).
Device layout: feature-major activations xT[d, t] (d on partitions, 6 chunks
of 128), bf16 matmul operands, fp32 residual. Scores are computed transposed
(sT[j,i]) so the softmax reduction becomes a ones-matmul and no on-chip
transposes are needed anywhere. LN gamma/beta and layer-scale are folded into
weights on the host; the adapter expert gather happens on the host during
sharding (it is per-sample indexing, i.e. data movement, not compute).
"""

import os
import sys

sys.path.insert(0, "/opt/trn_rl_repo")
sys.path.insert(0, "/root/.axon_site/_ro/trn_rl_repo")

from contextlib import ExitStack

import numpy as np
import ml_dtypes
from einops import rearrange

import concourse.bass as bass
import concourse.tile as tile
import concourse.mybir as mybir
from concourse import bacc
from concourse import bass_utils as _bu
from concourse.bass_utils import run_bass_kernel_spmd

# Restrict the ACT table sets to the two this kernel needs
# (natural_log_exp_and_others for LN rstd + softmax, gelu_and_others for MLP).
# With the full catalog, walrus bounces through extra sets on Square/Copy ops
# and the kernel pays ~139 table loads instead of ~26 (2.7us each, and they
# stall psum evictions). Graceful fallback to the stock file on any error.
def _setup_act_tables():
    try:
        import glob
        import json
        import tempfile
        from neuronxcc.driver.Job import Job

        cands = glob.glob(os.path.join(Job.getPackageDir(), "pwp",
                                       "pwp_bin_trainium*", "act_info.json"))
        if not cands:
            return
        src = cands[0]
        with open(src) as f:
            d = json.load(f)
        keep = {"natural_log_exp_and_others", "gelu_and_others"}
        d["act_func_sets"] = [s for s in d["act_func_sets"] if s["name"] in keep]
        if len(d["act_func_sets"]) != 2:
            return
        dstdir = tempfile.mkdtemp(prefix="act_custom_")
        import shutil

        srcdir = os.path.dirname(src)
        for fn in os.listdir(srcdir):
            if fn.endswith((".bin", ".json")) and fn != os.path.basename(src):
                try:
                    os.symlink(os.path.join(srcdir, fn), os.path.join(dstdir, fn))
                except OSError:
                    shutil.copy(os.path.join(srcdir, fn), os.path.join(dstdir, fn))
        dst = os.path.join(dstdir, os.path.basename(src))
        with open(dst, "w") as f:
            json.dump(d, f)
        os.environ["BASS_ACT_ROOT_JSON_PATH"] = dst
    except Exception:
        pass


_setup_act_tables()

# walrus's --enable-ldw-opt dedupes *consecutive identical* stationary-weight
# loads; the matmul loops below are ordered so each lhsT tile is used by two
# back-to-back matmuls (tb pairs), which halves LDWEIGHTS traffic there.
if not getattr(_bu.subprocess, "_ldwopt_patched", False):
    _orig_check_call = _bu.subprocess.check_call

    def _cc(argv, *a, **kw):
        if isinstance(argv, list) and argv and "walrus" in str(argv[0]):
            argv = ["--enable-ldw-opt=true" if x == "--enable-ldw-opt=false" else x
                    for x in argv]
        return _orig_check_call(argv, *a, **kw)

    _bu.subprocess.check_call = _cc
    _bu.subprocess._ldwopt_patched = True

# bass pre-places the table loads itself (bacc.insert_act_table_loads) using
# hw_specs.get_activation_tables; filter it to the same two sets so the
# pre-placed act_func_set_ids match the trimmed act_info.json walrus sees.
if "BASS_ACT_ROOT_JSON_PATH" in os.environ:
    from concourse import hw_specs as _hw

    _KEEP_SETS = ("natural_log_exp_and_others", "gelu_and_others")
    _orig_gat = _hw.get_activation_tables

    def _gat(arch):
        d = _orig_gat(arch)
        f = {k: d[k] for k in d if k in _KEEP_SETS}
        return f if len(f) == 2 else d

    _hw.get_activation_tables = _gat
    bacc.get_activation_tables = _gat

F32 = mybir.dt.float32
BF16 = mybir.dt.bfloat16
FP8 = mybir.dt.float8e4
AF = mybir.ActivationFunctionType
ALU = mybir.AluOpType
BF = ml_dtypes.bfloat16
# fp8 weights measured: per-layer quantization error is systematic (fixed
# weights), accumulates linearly over 12 layers -> 6e-2 rel err. Keep bf16.
FP8_MLP = False
TPAD = 800           # 16-byte-aligned token stride for fp8 pair layouts
DRMM = mybir.MatmulPerfMode.DoubleRow

B, IMG, PP, CIN = 32, 224, 16, 3
D, H, L, A, E, DD, FF = 768, 12, 12, 6, 8, 64, 3072
G = IMG // PP         # 14
N = G * G + 1         # 197
HD = D // H           # 64
NCORES = 8
S = B // NCORES       # 4 samples per core
T = S * N             # 788 tokens per core
DC = D // 128         # 6 chunks
QKC = 12              # q(6) + k(6) feature chunks
FJ = FF // 128        # 24
PC = (CIN * PP * PP) // 128  # 18
NPATCH = G * G        # 196
TB = 2
TBW = T // TB         # 394
EPS = 1e-6

_CACHE = {}


def _f(x):
    return np.asarray(x, np.float32)


def _prep(inputs):
    """Host-side prep: im2col, LN/LS folds, expert gather, bf16 packs."""
    pw = _f(inputs["patch_w"]); pb = _f(inputs["patch_b"])
    cls = _f(inputs["cls_token"]); pos = _f(inputs["pos_embed"])
    l1g = _f(inputs["ln1_g"]); l1b = _f(inputs["ln1_b"])
    qkvw = _f(inputs["qkv_w"]); qkvb = _f(inputs["qkv_b"])
    pjw = _f(inputs["proj_w"]); pjb = _f(inputs["proj_b"])
    ls1 = _f(inputs["ls1"]); ls2 = _f(inputs["ls2"])
    l2g = _f(inputs["ln2_g"]); l2b = _f(inputs["ln2_b"])
    f1w = _f(inputs["fc1_w"]); f1b = _f(inputs["fc1_b"])
    f2w = _f(inputs["fc2_w"]); f2b = _f(inputs["fc2_b"])
    ng = _f(inputs["norm_g"]); nb = _f(inputs["norm_b"])
    adw = _f(inputs["ad_down_w"]); adb = _f(inputs["ad_down_b"])
    auw = _f(inputs["ad_up_w"]); aub = _f(inputs["ad_up_b"])
    eids = np.asarray(inputs["expert_ids"], np.int64)
    imgs = _f(inputs["inputs"])

    shared = {}
    F8 = ml_dtypes.float8_e4m3
    qw = qkvw[:, :, :D]; kw = qkvw[:, :, D:2 * D]; vw = qkvw[:, :, 2 * D:]
    wqk = np.concatenate([qw, kw], axis=2) * l1g[:, :, None]          # [L,768,1536]
    # fp8 DoubleRow pair layout: partition pc carries contraction rows
    # (g*256+pc, g*256+128+pc); weights pre-scaled x32, activations x8,
    # descale 1/256 at psum evict. Quantization error is softmax/residual
    # damped here (measured 1.0e-2 final rel err in the numpy replica).
    shared["wqk8"] = rearrange(wqk * 32.0, "l (g o pc) (j pj) -> l j pc g o pj",
                               o=2, pc=128, pj=128).astype(F8)
    qkb = np.einsum("ldk,ld->lk", np.concatenate([qw, kw], axis=2), l1b) + qkvb[:, :2 * D]
    shared["qkb"] = rearrange(qkb, "l (j pj) -> l pj j", pj=128).astype(np.float32)

    wv = vw * l1g[:, :, None]
    shared["wv8"] = rearrange(wv * 32.0, "l (g o pc) d -> l pc g o d",
                              o=2, pc=128).astype(F8)
    vb = np.einsum("ldk,ld->lk", vw, l1b) + qkvb[:, 2 * D:]           # [L,768]

    wproj = pjw * ls1[:, None, :]
    shared["wproj8"] = rearrange(wproj * 32.0, "l (g o pc) (j pj) -> l j pc g o pj",
                                 o=2, pc=128, pj=128).astype(F8)
    pbe = ls1 * (pjb + np.einsum("ldk,ld->lk", pjw, vb))
    shared["pbe"] = rearrange(pbe, "l (j pj) -> l pj j", pj=128).astype(np.float32)

    wfc1 = f1w * l2g[:, :, None]
    shared["wfc1"] = rearrange(wfc1, "l (c pc) (j pj) -> l j pc c pj", pc=128, pj=128).astype(BF)
    f1be = np.einsum("ldk,ld->lk", f1w, l2b) + f1b
    shared["f1be"] = rearrange(f1be, "l (j pj) -> l pj j", pj=128).astype(np.float32)

    wfc2 = f2w * ls2[:, None, :]
    shared["wfc2"] = rearrange(wfc2, "l (c pc) (j pj) -> l j pc c pj", pc=128, pj=128).astype(BF)
    f2be = ls2 * f2b
    shared["f2be"] = rearrange(f2be, "l (j pj) -> l pj j", pj=128).astype(np.float32)

    wpatch = pw.T  # [2304, 768]
    shared["wpatch"] = rearrange(wpatch, "(c pc) (j pj) -> c pc j pj", pc=128, pj=128).astype(BF)

    posb = pos[0].copy()                  # [197, 768]
    posb[1:] += pb[None, :]
    posb[0] += cls[0, 0]
    shared["posbias"] = rearrange(posb, "t (c pc) -> pc c t", pc=128).astype(np.float32)

    shared["wnorm"] = np.stack([
        rearrange(ng, "(c pc) -> pc c", pc=128),
        rearrange(nb, "(c pc) -> pc c", pc=128)], axis=-1).astype(np.float32)  # [128,6,2]

    flags = dict(
        has_pbe=bool(np.abs(pbe).max() > 0),
        has_f2be=bool(np.abs(f2be).max() > 0),
        has_qkb=bool(np.abs(qkb).max() > 0),
        has_adb=bool(np.abs(adb).max() > 0),
        has_aub=bool(np.abs(aub).max() > 0),
    )

    per_core = []
    for core in range(NCORES):
        sl = slice(core * S, (core + 1) * S)
        im = imgs[sl]
        xp = im.reshape(S, CIN, G, PP, G, PP).transpose(0, 2, 4, 1, 3, 5).reshape(
            S * NPATCH, CIN * PP * PP)
        xpT = rearrange(np.ascontiguousarray(xp.T), "(c pc) t -> c pc t", pc=128).astype(BF)
        eid = eids[sl]
        pc_map = {
            "xpT": xpT,                                              # [18,128,784]
            "adwg": rearrange(adw[:, eid], "a s (c pc) k -> a pc c s k", pc=128).astype(BF),
            "auwg": rearrange(auw[:, eid], "a s k d -> a k s d").astype(BF),   # [A,64,S,768]
            "adbg": rearrange(adb[:, eid], "a s k -> a k s").astype(np.float32),
            "aubg": rearrange(aub[:, eid], "a s (j pj) -> a s pj j", pj=128).astype(np.float32),
        }
        per_core.append(pc_map)
    return shared, per_core, flags


def _build(flags, n_layers=L, dbg=False):
    key = (tuple(sorted(flags.items())), n_layers, dbg)
    if key in _CACHE:
        return _CACHE[key]
    nc = bacc.Bacc("TRN2", target_bir_lowering=False, debug=False, num_devices=NCORES)

    def din(name, shape, dt):
        return nc.dram_tensor(name, list(shape), dt, kind="ExternalInput").ap()

    xpT_d = din("xpT", [PC, 128, S * NPATCH], BF16)
    posb_d = din("posbias", [128, DC, N], F32)
    wqk_d = din("wqk8", [L, QKC, 128, DC // 2, 2, 128], FP8)
    qkb_d = din("qkb", [L, 128, QKC], F32)
    wv_d = din("wv8", [L, 128, DC // 2, 2, D], FP8)
    wproj_d = din("wproj8", [L, DC, 128, DC // 2, 2, 128], FP8)
    pbe_d = din("pbe", [L, 128, DC], F32)
    wfc1_d = din("wfc1", [L, FJ, 128, DC, 128], BF16)
    f1be_d = din("f1be", [L, 128, FJ], F32)
    wfc2_d = din("wfc2", [L, DC, 128, FJ, 128], BF16)
    f2be_d = din("f2be", [L, 128, DC], F32)
    wpatch_d = din("wpatch", [PC, 128, DC, 128], BF16)
    wnorm_d = din("wnorm", [128, DC, 2], F32)
    adwg_d = din("adwg", [A, 128, DC, S, DD], BF16)
    auwg_d = din("auwg", [A, DD, S, D], BF16)
    adbg_d = din("adbg", [A, DD, S], F32)
    aubg_d = din("aubg", [A, S, 128, DC], F32)

    out_d = nc.dram_tensor("out", [S, D], F32, kind="ExternalOutput")
    if dbg:
        xdbg_d = nc.dram_tensor("xdbg", [128, DC, T], F32, kind="ExternalOutput").ap()

    scol = [slice(s * N, (s + 1) * N) for s in range(S)]
    tbcol = [slice(tb * TBW, (tb + 1) * TBW) for tb in range(TB)]
    jts = [(0, 128), (128, N)]

    with tile.TileContext(nc) as tc:
        with ExitStack() as ctx:
            per = ctx.enter_context(tc.tile_pool(name="per", bufs=1))
            wq_p = ctx.enter_context(tc.tile_pool(name="wq", bufs=3))
            wf2_p = ctx.enter_context(tc.tile_pool(name="wf2", bufs=2))
            xp_p = ctx.enter_context(tc.tile_pool(name="xp", bufs=2))
            ad_p = ctx.enter_context(tc.tile_pool(name="ad", bufs=1))
            bia_p = ctx.enter_context(tc.tile_pool(name="bia", bufs=2))
            st_p = ctx.enter_context(tc.tile_pool(name="st", bufs=3))
            stp4 = ctx.enter_context(tc.tile_pool(name="stp4", bufs=4))
            exp_p = ctx.enter_context(tc.tile_pool(name="exp", bufs=4))
            lno_p = ctx.enter_context(tc.tile_pool(name="lno", bufs=2))
            sq_p = ctx.enter_context(tc.tile_pool(name="sq", bufs=1))
            xbf_p = ctx.enter_context(tc.tile_pool(name="xbf", bufs=2))
            ps_mm = ctx.enter_context(tc.tile_pool(name="psmm", bufs=3, space="PSUM"))
            ps_at = ctx.enter_context(tc.tile_pool(name="psat", bufs=3, space="PSUM"))
            ps_sm = ctx.enter_context(tc.tile_pool(name="pssm", bufs=2, space="PSUM"))

            x = per.tile([128, DC, T], F32, tag="x")
            qk = per.tile([128, QKC, T], BF16, tag="qk")
            v_tok = per.tile([128, S, 2, D], BF16, tag="vtok")
            attn = per.tile([128, DC // 2, 2, TPAD], FP8, tag="attn")
            hml = per.tile([128, FJ, T], BF16, tag="hml")
            wv_t = per.tile([128, DC // 2, 2, D], FP8, tag="wvt")
            posb = per.tile([128, DC, N], F32, tag="posb")
            ones1 = per.tile([128, 1], BF16, tag="ones1")
            ones8 = per.tile([128, 1], BF16, tag="ones8")
            wnorm_t = per.tile([128, DC, 2], F32, tag="wnormt")

            nc.vector.memset(ones1[:], 1.0)
            nc.vector.memset(ones8[:], 0.125)
            czero = per.tile([128, 1], F32, tag="czero")
            nc.vector.memset(czero[:], 0.0)
            ceps = per.tile([128, 1], F32, tag="ceps")
            nc.vector.memset(ceps[:], EPS)
            clog8 = per.tile([128, 1], F32, tag="clog8")
            nc.vector.memset(clog8[:], float(np.log(8.0)))
            nc.const_aps.aps[(F32, 0.0)] = czero[:]
            nc.const_aps.aps[(F32, EPS)] = ceps[:]
            nc.const_aps.aps[(F32, float(np.log(8.0)))] = clog8[:]
            nc.sync.dma_start(out=posb[:], in_=posb_d[:])
            nc.sync.dma_start(out=wnorm_t[:], in_=wnorm_d[:])

            # ======== patch embed ========
            # c-outer / j-inner: each xp chunk is DMA'd once per sample-half
            # and feeds all 6 output chunks held in 6 live psum banks
            # (3 from ps_mm + 3 from ps_at; nothing else uses psum yet).
            x_bf = xbf_p.tile([128, DC, T], BF16, tag="xbf")
            for sh in range(2):
                ps6 = []
                for j in range(DC):
                    psj = (ps_mm if j < 3 else ps_at).tile(
                        [128, 2 * NPATCH], F32, tag=("mm" if j < 3 else "at"),
                        name=f"pspe{j}")
                    ps6.append(psj)
                for c in range(PC):
                    wpc = wq_p.tile([128, DC, 128], BF16, tag="wq")
                    nc.sync.dma_start(out=wpc[:], in_=wpatch_d[c])
                    xpc = xp_p.tile([128, 2 * NPATCH], BF16, tag="xp")
                    nc.sync.dma_start(
                        out=xpc[:],
                        in_=xpT_d[c, :, sh * 2 * NPATCH:(sh + 1) * 2 * NPATCH])
                    for j in range(DC):
                        nc.tensor.matmul(ps6[j][:], wpc[:, j, :], xpc[:],
                                         start=(c == 0), stop=(c == PC - 1))
                for j in range(DC):
                    for si in range(2):
                        s = sh * 2 + si
                        nc.vector.tensor_tensor(
                            x[:, j, s * N + 1:(s + 1) * N],
                            ps6[j][:, si * NPATCH:(si + 1) * NPATCH],
                            posb[:, j, 1:N], ALU.add)
                        nc.vector.tensor_copy(x_bf[:, j, s * N + 1:(s + 1) * N],
                                              x[:, j, s * N + 1:(s + 1) * N])
            for j in range(DC):
                for s in range(S):
                    nc.vector.tensor_copy(x[:, j, s * N:s * N + 1], posb[:, j, 0:1])
                    nc.vector.tensor_copy(x_bf[:, j, s * N:s * N + 1], posb[:, j, 0:1])

            LOG8 = float(np.log(8.0))

            def layernorm_stats_tb(src_bf, sqt, tb, scale8=False):
                """One token-block's LN stats -> broadcast ab tile [128,2,TBW].
                Emitted at the tail of the producing matmul stage (proj/fc2)
                so the rstd scalar/vector/gpsimd chain overlaps the other
                token-block's PE work. scale8 folds the x8 fp8 headroom
                factor into ab (ln1 is stored fp8 at 8x scale)."""
                for c in range(DC):
                    nc.scalar.activation(sqt[:, c, :], src_bf[:, c, tbcol[tb]], AF.Square)
                sm_x = ps_sm.tile([1, TBW], F32, tag="sm")
                sm_q = ps_sm.tile([1, TBW], F32, tag="sm")
                for c in range(DC):
                    nc.tensor.matmul(sm_x[:], ones1[:], src_bf[:, c, tbcol[tb]],
                                     start=(c == 0), stop=(c == DC - 1))
                for c in range(DC):
                    nc.tensor.matmul(sm_q[:], ones1[:], sqt[:, c, :],
                                     start=(c == 0), stop=(c == DC - 1))
                # mA = mean; varD = Sum(x^2) - Sum(x)^2/D; r = (varD/D + eps)^-1/2
                mA = stp4.tile([1, TBW], F32, tag="stat")
                nc.vector.tensor_scalar_mul(mA[:], sm_x[:], 1.0 / D)
                msqD = stp4.tile([1, TBW], F32, tag="stat")
                nc.vector.tensor_tensor(msqD[:], mA[:], sm_x[:], ALU.mult)
                varD = stp4.tile([1, TBW], F32, tag="stat")
                nc.vector.tensor_tensor(varD[:], sm_q[:], msqD[:], ALU.subtract)
                r = stp4.tile([1, TBW], F32, tag="stat")
                nc.scalar.activation(r[:], varD[:], AF.Ln, bias=EPS, scale=1.0 / D)
                ab = st_p.tile([1, 2, TBW], BF16, tag="ab")
                nc.scalar.activation(ab[:, 0, :], r[:], AF.Exp, scale=-0.5,
                                     bias=LOG8 if scale8 else 0.0)
                nc.vector.scalar_tensor_tensor(ab[:, 1, :], mA[:], -1.0,
                                               ab[:, 0, :], ALU.mult, ALU.mult)
                abb = st_p.tile([128, 2, TBW], BF16, tag="abb")
                nc.gpsimd.partition_broadcast(abb[:], ab[0:1, :, :])
                return abb

            def ln_apply_tb(src_bf, abb, dst, tb, f8=False):
                for c in range(DC):
                    if f8:
                        tmp = stp4.tile([128, TBW], BF16, tag="lntmp", bufs=2)
                        nc.vector.tensor_tensor(tmp[:], src_bf[:, c, tbcol[tb]],
                                                abb[:, 0, :], ALU.mult)
                        nc.vector.tensor_tensor(dst[:, c // 2, c % 2, tbcol[tb]],
                                                tmp[:], abb[:, 1, :], ALU.add)
                    else:
                        nc.vector.tensor_tensor(dst[:, c, tbcol[tb]],
                                                src_bf[:, c, tbcol[tb]],
                                                abb[:, 0, :], ALU.mult)
                        nc.vector.tensor_tensor(dst[:, c, tbcol[tb]],
                                                dst[:, c, tbcol[tb]],
                                                abb[:, 1, :], ALU.add)

            # ======== transformer layers ========
            ln1_t = None
            for l in range(n_layers):
                # ---- LN1 ---- (x_bf shadow-written by patch/fc2 evicts;
                # stats+apply for l>0 were emitted at the tail of fc2)
                if ln1_t is None:
                    sqt = sq_p.tile([128, DC, TBW], BF16, tag="sq", name="sqt0")
                    ln1 = lno_p.tile([128, DC // 2, 2, TPAD], FP8, tag="lnof8",
                                     name="ln1_0", bufs=1)
                    for tb in range(TB):
                        abb0 = layernorm_stats_tb(x_bf, sqt, tb, scale8=True)
                        ln_apply_tb(x_bf, abb0, ln1, tb, f8=True)
                else:
                    ln1 = ln1_t

                # ---- QK ---- staggered: tb0 chains run 2 j ahead of tb1 so
                # the PE never sits behind the LN1(tb1) apply; q/k chunk pairs
                # emitted together so attention head-pair a can start early.
                if flags["has_qkb"]:
                    qkb_t = bia_p.tile([128, QKC], F32, tag="qkb")
                    nc.sync.dma_start(out=qkb_t[:], in_=qkb_d[l])

                def qk_evict(j, tb, ps):
                    # descale the x8-act x32-weight fp8 product
                    if flags["has_qkb"]:
                        nc.scalar.activation(qk[:, j, tbcol[tb]], ps[:], AF.Identity,
                                             scale=1.0 / 256, bias=qkb_t[:, j:j + 1])
                    else:
                        nc.scalar.mul(qk[:, j, tbcol[tb]], ps[:], 1.0 / 256)

                for j in [0, 6, 1, 7, 2, 8, 3, 9, 4, 10, 5, 11]:
                    wj = wq_p.tile([128, DC // 2, 2, 128], FP8, tag="wq")
                    nc.sync.dma_start(out=wj[:], in_=wqk_d[l, j])
                    ps0 = ps_mm.tile([128, TBW], F32, tag="mm")
                    ps1 = ps_mm.tile([128, TBW], F32, tag="mm")
                    for g in range(DC // 2):  # same lhsT twice -> ldw-opt dedupes
                        nc.tensor.matmul(ps0[:], wj[:, g, :, :],
                                         ln1[:, g, :, tbcol[0]],
                                         start=(g == 0), stop=(g == DC // 2 - 1),
                                         perf_mode=DRMM)
                        nc.tensor.matmul(ps1[:], wj[:, g, :, :],
                                         ln1[:, g, :, tbcol[1]],
                                         start=(g == 0), stop=(g == DC // 2 - 1),
                                         perf_mode=DRMM)
                    qk_evict(j, 0, ps0)
                    qk_evict(j, 1, ps1)

                # ---- V (token-major) ----
                nc.sync.dma_start(out=wv_t[:], in_=wv_d[l])
                for s in range(S):
                    for jt, (j0, j1) in enumerate(jts):
                        tn = j1 - j0
                        ps0 = ps_mm.tile([128, 384], F32, tag="mm")
                        ps1 = ps_mm.tile([128, 384], F32, tag="mm")
                        for c in range(DC):
                            lh = ln1[:, c // 2, c % 2, s * N + j0:s * N + j1]
                            nc.tensor.matmul(ps0[:tn, :], lh,
                                             wv_t[:, c // 2, c % 2, 0:384],
                                             start=(c == 0), stop=(c == DC - 1))
                            nc.tensor.matmul(ps1[:tn, :], lh,
                                             wv_t[:, c // 2, c % 2, 384:768],
                                             start=(c == 0), stop=(c == DC - 1))
                        nc.vector.tensor_scalar_mul(v_tok[:tn, s, jt, 0:384],
                                                    ps0[:tn, :], 1.0 / 256)
                        nc.vector.tensor_scalar_mul(v_tok[:tn, s, jt, 384:768],
                                                    ps1[:tn, :], 1.0 / 256)

                # ---- attention ---- software-pipelined over (s, head-pair).
                # Stages: st0 scores+exp -> st1 ones-sum+recip+bcast -> st2
                # attnV -> st3 evict. Lookahead-2 emission keeps independent
                # score streams in front of the PE queue while the current
                # item's exp/recip/broadcast chain resolves on the other
                # engines.
                items = [(s, a) for s in range(S) for a in range(DC)]
                at_exp, at_rcb, at_ps = {}, {}, {}

                def att_st0(it):
                    s, a = it
                    sT_e = ps_at.tile([128, 2, N], F32, tag="at")
                    sT_o = ps_at.tile([128, 2, N], F32, tag="at")
                    # jt0+jt1 chained into one accumulation group per psum
                    # tile (disjoint halves, start=True on both) -> one
                    # group-end drain instead of two.
                    for sT, p0, tp in ((sT_e, 0, (0, 0)), (sT_o, 64, (64, 0))):
                        for jt, (j0, j1) in enumerate(jts):
                            tn = j1 - j0
                            nc.tensor.matmul(
                                sT[:tn, jt, :],
                                qk[p0:p0 + 64, DC + a, scol[s]][:, j0:j1],
                                qk[p0:p0 + 64, a, scol[s]],
                                start=True, stop=(jt == 1),
                                skip_group_check=(jt == 1),
                                tile_position=tp)
                    expe = exp_p.tile([128, 2, N], BF16, tag="exp")
                    expo = exp_p.tile([128, 2, N], BF16, tag="exp")
                    # one wide exp per head; rows 69-127 of the jt=1 slice are
                    # garbage (never read by the cs/oT matmuls below)
                    nc.scalar.activation(expe[:], sT_e[:], AF.Exp, scale=0.125)
                    nc.scalar.activation(expo[:], sT_o[:], AF.Exp, scale=0.125)
                    at_exp[it] = (expe, expo)

                def att_st1(it):
                    expe, expo = at_exp[it]
                    cs = ps_sm.tile([1, 2, N], F32, tag="sm")
                    nc.tensor.matmul(cs[:, 0, :], ones8[:], expe[:, 0, :],
                                     start=True, stop=False)
                    nc.tensor.matmul(cs[:, 0, :], ones8[:69, :], expe[:69, 1, :],
                                     start=False, stop=False)
                    nc.tensor.matmul(cs[:, 1, :], ones8[:], expo[:, 0, :],
                                     start=True, stop=False, skip_group_check=True)
                    nc.tensor.matmul(cs[:, 1, :], ones8[:69, :], expo[:69, 1, :],
                                     start=False, stop=True)
                    rec = st_p.tile([1, 2, N], F32, tag="rec", bufs=2)
                    nc.vector.reciprocal_approx_fast(rec[:], cs[:])
                    rcb = st_p.tile([128, 2, N], F32, tag="rcb", bufs=2)
                    nc.gpsimd.partition_broadcast(rcb[:], rec[0:1, :, :])
                    at_rcb[it] = rcb

                def att_st2(it):
                    s, a = it
                    expe, expo = at_exp[it]
                    psA = ps_mm.tile([128, N], F32, tag="mm")
                    psB = ps_mm.tile([128, N], F32, tag="mm")
                    dlo = a * 128
                    nc.tensor.matmul(psA[:], v_tok[:, s, 0, dlo:dlo + 128], expe[:, 0, :],
                                     start=True, stop=False)
                    nc.tensor.matmul(psB[:], v_tok[:, s, 0, dlo:dlo + 128], expo[:, 0, :],
                                     start=True, stop=False)
                    nc.tensor.matmul(psA[:], v_tok[:69, s, 1, dlo:dlo + 128],
                                     expe[:69, 1, :], start=False, stop=True)
                    nc.tensor.matmul(psB[:], v_tok[:69, s, 1, dlo:dlo + 128],
                                     expo[:69, 1, :], start=False, stop=True)
                    at_ps[it] = (psA, psB)

                def att_st3(it):
                    s, a = it
                    psA, psB = at_ps.pop(it)
                    rcb = at_rcb.pop(it)
                    at_exp.pop(it)
                    nc.vector.tensor_tensor(attn[0:64, a // 2, a % 2, scol[s]],
                                            psA[0:64, :], rcb[0:64, 0, :], ALU.mult)
                    nc.vector.tensor_tensor(attn[64:128, a // 2, a % 2, scol[s]],
                                            psB[64:128, :], rcb[64:128, 1, :], ALU.mult)

                att_st0(items[0])
                att_st0(items[1])
                att_st1(items[0])
                for i in range(len(items)):
                    if i + 2 < len(items):
                        att_st0(items[i + 2])
                    if i + 1 < len(items):
                        att_st1(items[i + 1])
                    att_st2(items[i])
                    att_st3(items[i])

                # Last layer: only the CLS columns survive to the final LN, so
                # proj/LN2/MLP run on 4 columns instead of 788.
                last = (l == n_layers - 1) and (not dbg) and l >= A
                if last:
                    if flags["has_pbe"]:
                        pbe_t = bia_p.tile([128, DC], F32, tag="pbe")
                        nc.sync.dma_start(out=pbe_t[:], in_=pbe_d[l])
                    for j in range(DC):
                        wj = wq_p.tile([128, DC // 2, 2, 128], FP8, tag="wq")
                        nc.sync.dma_start(out=wj[:], in_=wproj_d[l, j])
                        ps = ps_mm.tile([128, S], F32, tag="mm")
                        for g in range(DC // 2):
                            nc.tensor.matmul(ps[:], wj[:, g, :, :],
                                             attn[:, g, :, 0:T:N],
                                             start=(g == 0), stop=(g == DC // 2 - 1),
                                             perf_mode=DRMM)
                        nc.vector.scalar_tensor_tensor(
                            x[:, j, ::N], ps[:], 1.0 / 256,
                            x[:, j, ::N], ALU.mult, ALU.add)
                        if flags["has_pbe"]:
                            nc.vector.tensor_scalar_add(x[:, j, ::N], x[:, j, ::N],
                                                        pbe_t[:, j:j + 1])
                    # LN2 on CLS columns
                    xbfc = st_p.tile([128, DC, S], BF16, tag="xbfc")
                    for c in range(DC):
                        nc.vector.tensor_copy(xbfc[:, c, :], x[:, c, ::N])
                    sqc2 = st_p.tile([128, DC, S], BF16, tag="sqc2")
                    nc.scalar.activation(sqc2[:], xbfc[:], AF.Square)
                    smc_x = ps_sm.tile([1, S], F32, tag="sm")
                    smc_q = ps_sm.tile([1, S], F32, tag="sm")
                    for c in range(DC):
                        nc.tensor.matmul(smc_x[:], ones1[:], xbfc[:, c, :],
                                         start=(c == 0), stop=(c == DC - 1))
                    for c in range(DC):
                        nc.tensor.matmul(smc_q[:], ones1[:], sqc2[:, c, :],
                                         start=(c == 0), stop=(c == DC - 1))
                    mAc = st_p.tile([1, S], F32, tag="mAc")
                    nc.vector.tensor_scalar_mul(mAc[:], smc_x[:], 1.0 / D)
                    msqc = st_p.tile([1, S], F32, tag="msqc")
                    nc.vector.tensor_tensor(msqc[:], mAc[:], smc_x[:], ALU.mult)
                    varc = st_p.tile([1, S], F32, tag="varc")
                    nc.vector.tensor_tensor(varc[:], smc_q[:], msqc[:], ALU.subtract)
                    rc = st_p.tile([1, S], F32, tag="rcl")
                    nc.scalar.activation(rc[:], varc[:], AF.Ln, bias=EPS, scale=1.0 / D)
                    abc = st_p.tile([1, 2, S], BF16, tag="abc")
                    nc.scalar.activation(abc[:, 0, :], rc[:], AF.Exp, scale=-0.5)
                    mrc = st_p.tile([1, S], F32, tag="mrc")
                    nc.vector.tensor_tensor(mrc[:], mAc[:], abc[:, 0, :], ALU.mult)
                    nc.vector.tensor_scalar_mul(abc[:, 1, :], mrc[:], -1.0)
                    abbc = st_p.tile([128, 2, S], BF16, tag="abbc")
                    nc.gpsimd.partition_broadcast(abbc[:], abc[0:1, :, :])
                    ln2c = st_p.tile([128, DC, S], BF16, tag="ln2c")
                    for c in range(DC):
                        nc.vector.tensor_tensor(ln2c[:, c, :], xbfc[:, c, :],
                                                abbc[:, 0, :], ALU.mult)
                        nc.vector.tensor_tensor(ln2c[:, c, :], ln2c[:, c, :],
                                                abbc[:, 1, :], ALU.add)
                    # MLP on CLS columns
                    f1be_t = bia_p.tile([128, FJ], F32, tag="f1b")
                    nc.sync.dma_start(out=f1be_t[:], in_=f1be_d[l])
                    if flags["has_f2be"]:
                        f2be_t = bia_p.tile([128, DC], F32, tag="f2b")
                        nc.sync.dma_start(out=f2be_t[:], in_=f2be_d[l])
                    hc = st_p.tile([128, FJ, S], BF16, tag="hc")
                    for j in range(FJ):
                        wj = wq_p.tile([128, DC, 128], BF16, tag="wq")
                        nc.sync.dma_start(out=wj[:], in_=wfc1_d[l, j])
                        ps = ps_mm.tile([128, S], F32, tag="mm")
                        for c in range(DC):
                            nc.tensor.matmul(ps[:], wj[:, c, :], ln2c[:, c, :],
                                             start=(c == 0), stop=(c == DC - 1))
                        nc.scalar.activation(hc[:, j, :], ps[:], AF.Gelu,
                                             bias=f1be_t[:, j:j + 1])
                    for j in range(DC):
                        w2j = wf2_p.tile([128, FJ, 128], BF16, tag="wf2")
                        nc.sync.dma_start(out=w2j[:], in_=wfc2_d[l, j])
                        ps = ps_mm.tile([128, S], F32, tag="mm")
                        for c in range(FJ):
                            nc.tensor.matmul(ps[:], w2j[:, c, :], hc[:, c, :],
                                             start=(c == 0), stop=(c == FJ - 1))
                        if flags["has_f2be"]:
                            nc.vector.scalar_tensor_tensor(
                                x[:, j, ::N], ps[:], f2be_t[:, j:j + 1],
                                x[:, j, ::N], ALU.add, ALU.add)
                        else:
                            nc.vector.tensor_tensor(x[:, j, ::N], ps[:],
                                                    x[:, j, ::N], ALU.add)
                    continue

                # ---- proj + residual (shadow bf16 for LN2/adapter) ----
                x_bf2 = xbf_p.tile([128, DC, T], BF16, tag="xbf")
                if flags["has_pbe"]:
                    pbe_t = bia_p.tile([128, DC], F32, tag="pbe")
                    nc.sync.dma_start(out=pbe_t[:], in_=pbe_d[l])
                # tb-outer so LN2 stats(tb0) + rstd + apply(tb0) overlap
                # proj(tb1) PE work (costs a second wproj DMA pass, 1.2MB).
                sqt2 = sq_p.tile([128, DC, TBW], BF16, tag="sq")
                ln2 = lno_p.tile([128, DC, T], BF16, tag="lno")
                for tb in range(TB):
                    for j in range(DC):
                        wj = wq_p.tile([128, DC // 2, 2, 128], FP8, tag="wq")
                        nc.sync.dma_start(out=wj[:], in_=wproj_d[l, j])
                        ps = ps_mm.tile([128, TBW], F32, tag="mm")
                        for g in range(DC // 2):
                            nc.tensor.matmul(ps[:], wj[:, g, :, :],
                                             attn[:, g, :, tbcol[tb]],
                                             start=(g == 0), stop=(g == DC // 2 - 1),
                                             perf_mode=DRMM)
                        nc.vector.scalar_tensor_tensor(
                            x[:, j, tbcol[tb]], ps[:], 1.0 / 256,
                            x[:, j, tbcol[tb]], ALU.mult, ALU.add)
                        if flags["has_pbe"]:
                            nc.vector.tensor_scalar_add(
                                x[:, j, tbcol[tb]], x[:, j, tbcol[tb]],
                                pbe_t[:, j:j + 1])
                        nc.vector.tensor_copy(x_bf2[:, j, tbcol[tb]],
                                              x[:, j, tbcol[tb]])
                    abb2 = layernorm_stats_tb(x_bf2, sqt2, tb)
                    ln_apply_tb(x_bf2, abb2, ln2, tb)

                # ---- adapter ----
                if l < A:
                    adw_t = ad_p.tile([128, DC, S, DD], BF16, tag="adw")
                    auw_t = ad_p.tile([DD, S, D], BF16, tag="auw")
                    nc.sync.dma_start(out=adw_t[:], in_=adwg_d[l])
                    nc.sync.dma_start(out=auw_t[:], in_=auwg_d[l])
                    if flags["has_adb"]:
                        adbg_t = bia_p.tile([DD, S], F32, tag="adb")
                        nc.sync.dma_start(out=adbg_t[:], in_=adbg_d[l])
                    if flags["has_aub"]:
                        aubg_t = bia_p.tile([S, 128, DC], F32, tag="aub")
                        nc.sync.dma_start(out=aubg_t[:], in_=aubg_d[l])
                def adapter_compute(adw_t=None, auw_t=None, x_src=None,
                                    adbg=None, aubg=None):
                    # psums come from ps_at (idle during the MLP); gelus are
                    # emitted mid-fc1 so they join the gelu table-set run
                    # instead of thrashing against LN exp/ln loads.
                    for s in range(S):
                        psh = ps_at.tile([DD, N], F32, tag="at")
                        for c in range(DC):
                            nc.tensor.matmul(psh[:], adw_t[:, c, s, :], x_src[:, c, scol[s]],
                                             start=(c == 0), stop=(c == DC - 1))
                        hp = st_p.tile([DD, N], BF16, tag="hp")
                        if adbg is not None:
                            nc.scalar.activation(hp[:], psh[:], AF.Gelu,
                                                 bias=adbg[:, s:s + 1])
                        else:
                            nc.scalar.activation(hp[:], psh[:], AF.Gelu)
                        for j in range(DC):
                            psu = ps_at.tile([128, N], F32, tag="at")
                            nc.tensor.matmul(psu[:], auw_t[:, s, j * 128:(j + 1) * 128],
                                             hp[:], start=True, stop=True)
                            if aubg is not None:
                                nc.vector.scalar_tensor_tensor(
                                    x[:, j, scol[s]], psu[:], aubg[s, :, j:j + 1],
                                    x[:, j, scol[s]], ALU.add, ALU.add)
                            else:
                                nc.vector.tensor_tensor(x[:, j, scol[s]], psu[:],
                                                        x[:, j, scol[s]], ALU.add)

                if l < A:
                    ad_args = dict(adw_t=adw_t, auw_t=auw_t, x_src=x_bf2,
                                   adbg=adbg_t if flags["has_adb"] else None,
                                   aubg=aubg_t if flags["has_aub"] else None)

                # ---- MLP ---- (fc2 evicts shadow next layer's LN1 input)
                x_bf = xbf_p.tile([128, DC, T], BF16, tag="xbf")
                f1be_t = bia_p.tile([128, FJ], F32, tag="f1b")
                nc.sync.dma_start(out=f1be_t[:], in_=f1be_d[l])
                if flags["has_f2be"]:
                    f2be_t = bia_p.tile([128, DC], F32, tag="f2b")
                    nc.sync.dma_start(out=f2be_t[:], in_=f2be_d[l])
                for j in range(FJ):
                    wj = wq_p.tile([128, DC, 128], BF16, tag="wq")
                    nc.sync.dma_start(out=wj[:], in_=wfc1_d[l, j])
                    ps0 = ps_mm.tile([128, TBW], F32, tag="mm")
                    ps1 = ps_mm.tile([128, TBW], F32, tag="mm")
                    for c in range(DC):
                        nc.tensor.matmul(ps0[:], wj[:, c, :], ln2[:, c, tbcol[0]],
                                         start=(c == 0), stop=(c == DC - 1))
                        nc.tensor.matmul(ps1[:], wj[:, c, :], ln2[:, c, tbcol[1]],
                                         start=(c == 0), stop=(c == DC - 1))
                    nc.scalar.activation(hml[:, j, tbcol[0]], ps0[:], AF.Gelu,
                                         bias=f1be_t[:, j:j + 1])
                    nc.scalar.activation(hml[:, j, tbcol[1]], ps1[:], AF.Gelu,
                                         bias=f1be_t[:, j:j + 1])
                    if j == 5 and l < A:
                        adapter_compute(**ad_args)
                # fc2 tb-outer: LN1(l+1) stats+apply for each tb run right
                # after its last evict; the rstd chain overlaps the other
                # tb's (or QK's) PE work.
                sqt_n = sq_p.tile([128, DC, TBW], BF16, tag="sq", name=f"sqtn{l}")
                ln1_t = lno_p.tile([128, DC // 2, 2, TPAD], FP8, tag="lnof8",
                                   name=f"ln1n{l}", bufs=1)
                for tb in range(TB):
                    for j in range(DC):
                        w2j = wf2_p.tile([128, FJ, 128], BF16, tag="wf2",
                                         name=f"w2j{tb}_{j}")
                        nc.sync.dma_start(out=w2j[:], in_=wfc2_d[l, j])
                        ps = ps_mm.tile([128, TBW], F32, tag="mm")
                        for c in range(FJ):
                            nc.tensor.matmul(ps[:], w2j[:, c, :], hml[:, c, tbcol[tb]],
                                             start=(c == 0), stop=(c == FJ - 1))
                        if flags["has_f2be"]:
                            nc.vector.scalar_tensor_tensor(
                                x[:, j, tbcol[tb]], ps[:], f2be_t[:, j:j + 1],
                                x[:, j, tbcol[tb]], ALU.add, ALU.add)
                        else:
                            nc.vector.tensor_tensor(x[:, j, tbcol[tb]], ps[:],
                                                    x[:, j, tbcol[tb]], ALU.add)
                        nc.vector.tensor_copy(x_bf[:, j, tbcol[tb]],
                                              x[:, j, tbcol[tb]])
                    abbn = layernorm_stats_tb(x_bf, sqt_n, tb, scale8=True)
                    ln_apply_tb(x_bf, abbn, ln1_t, tb, f8=True)

            if dbg:
                for c in range(DC):
                    nc.sync.dma_start(out=xdbg_d[:, c, :], in_=x[:, c, :])

            # ======== final LN on CLS columns + output ========
            xc = st_p.tile([128, DC, S], F32, tag="xc")
            for c in range(DC):
                nc.vector.tensor_copy(xc[:, c, :], x[:, c, ::N])
            xcb = st_p.tile([128, DC, S], BF16, tag="xcb")
            nc.vector.tensor_copy(xcb[:], xc[:])
            sqc = st_p.tile([128, DC, S], BF16, tag="sqc")
            nc.scalar.activation(sqc[:], xcb[:], AF.Square)
            fs_x = ps_sm.tile([1, S], F32, tag="sm")
            fs_q = ps_sm.tile([1, S], F32, tag="sm")
            for c in range(DC):
                nc.tensor.matmul(fs_x[:], ones1[:], xcb[:, c, :], start=(c == 0),
                                 stop=(c == DC - 1))
            for c in range(DC):
                nc.tensor.matmul(fs_q[:], ones1[:], sqc[:, c, :], start=(c == 0),
                                 stop=(c == DC - 1))
            fmean = st_p.tile([1, S], F32, tag="fmean")
            nc.vector.tensor_scalar_mul(fmean[:], fs_x[:], 1.0 / D)
            var = st_p.tile([1, S], F32, tag="fvar")
            nc.vector.tensor_scalar_mul(var[:], fs_q[:], 1.0 / D)
            fmsq = st_p.tile([1, S], F32, tag="fmsq")
            nc.vector.tensor_tensor(fmsq[:], fmean[:], fmean[:], ALU.mult)
            nc.vector.tensor_tensor(var[:], var[:], fmsq[:], ALU.subtract)
            r = st_p.tile([1, S], F32, tag="fr")
            nc.scalar.activation(r[:], var[:], AF.Ln, bias=EPS)
            nc.scalar.activation(r[:], r[:], AF.Exp, scale=-0.5)
            rb = st_p.tile([128, S], F32, tag="frb")
            nc.gpsimd.partition_broadcast(rb[:], r[:])
            mb = st_p.tile([128, S], F32, tag="fmb")
            nc.gpsimd.partition_broadcast(mb[:], fmean[:])
            on = st_p.tile([128, DC, S], F32, tag="on")
            for c in range(DC):
                nc.vector.tensor_tensor(on[:, c, :], xc[:, c, :], mb[:], ALU.subtract)
                nc.vector.tensor_tensor(on[:, c, :], on[:, c, :], rb[:], ALU.mult)
                nc.vector.tensor_scalar(on[:, c, :], on[:, c, :],
                                        wnorm_t[:, c, 0:1], wnorm_t[:, c, 1:2],
                                        ALU.mult, ALU.add)
            for c in range(DC):
                dst = bass.AP(tensor=out_d, offset=c * 128, ap=[[1, 128], [D, S]])
                nc.sync.dma_start(out=dst, in_=on[:, c, :])

    nc.compile()
    _CACHE[key] = nc
    return nc


def kernel(_n_layers=L, _dbg=False, **inputs):
    shared, per_core, flags = _prep(inputs)
    nc = _build(flags, n_layers=_n_layers, dbg=_dbg)
    in_maps = []
    for core in range(NCORES):
        m = dict(shared)
        m.update(per_core[core])
        in_maps.append(m)
    try:
        res = run_bass_kernel_spmd(nc, in_maps, core_ids=list(range(NCORES)))
    except Exception:
        # transient NRT faults have been observed once; one retry
        res = run_bass_kernel_spmd(nc, in_maps, core_ids=list(range(NCORES)))
    out = np.concatenate([res.results[i]["out"] for i in range(NCORES)], axis=0)
    if _dbg:
        xd = [res.results[i]["xdbg"] for i in range(NCORES)]
        return out.astype(np.float32), xd
    return out.astype(np.float32)

